# revision 13
# baseline (speedup 1.0000x reference)
"""Cross-attention kernel for Trainium2, sharded over 8 NeuronCores.

Problem (per reference):
  q = wq @ x_q + bq ; k = wk @ x_kv + bk ; v = wv @ x_kv + bv   (1x1 convs)
  per head: attn = softmax(q^T k / sqrt(hd)) ; out = attn @ v^T
  y = wo @ out + bo

Sharding: core c -> (batch b = c // 4, head n = c % 4). Each core runs one
head's full attention and produces the partial output projection
y_part = wo[:, head] @ out_head; the host sums the 4 head partials per batch.

Device-side simplifications (all mathematically exact):
  * bk drops out entirely: a per-query constant shift of the logits cancels
    in softmax.
  * bv folds into the output bias: sum_j softmax_ij = 1, so v-bias
    contributes wo_col @ bv, added to bo on the host.
  * scale 1/8 folds into wq/bq on the host.
  * no max-subtraction: logits are ~N(0,1) (max |logit| < ~6), exp is safe
    in fp32.
  * softmax denominator comes from a ones-column appended to v^T in the AV
    matmul; normalization happens after AV on [64, S] instead of [S, S].

Layouts: logits are computed transposed, S^T[j, i] (k stationary, q moving),
so the exp'd tile feeds the AV matmul directly with j on partitions — no
transposes anywhere. v^T is produced directly by using x_kv chunks as the
stationary operand of the v projection.
"""

import numpy as np
import ml_dtypes

import concourse.bacc as bacc
import concourse.mybir as mybir
import concourse.tile as tile
from concourse.bass_utils import run_bass_kernel_spmd

F32 = mybir.dt.float32
BF16 = mybir.dt.bfloat16

B, C, HGT, WID = 2, 256, 64, 64
S = HGT * WID  # 4096 pixels
NH, HD = 4, 64
NCORES = 8
P = 128
IC = 1024  # i-chunk width (2 PSUM banks)
NI = S // IC  # 4
NJ = S // P  # 32 j-blocks
SCALE = HD ** -0.5
EXP_W = 1024  # free width of one exp instruction (must divide IC)
# Width of the dummy matmul issued each j-iteration into the unused
# partitions of the AV PSUM tile. The attention loop is ACT(exp)-bound, so
# the PE idles ~25% per iteration; the hardware activity monitor then
# down-clocks it to 1.2 GHz, at which point it becomes the bottleneck and
# stays throttled (measured: stuck at K=4/8 for 90% of the run). Padding
# the PE's duty cycle with a throwaway matmul keeps it at 2.4 GHz.
JUNK_W = 512


def _emit(tc):
    nc = tc.nc
    xq = nc.dram_tensor("xq", [2, P, S], BF16, kind="ExternalInput").ap()
    xkv = nc.dram_tensor("xkv", [2, P, S], BF16, kind="ExternalInput").ap()
    wqT = nc.dram_tensor("wqT", [2, P, HD], BF16, kind="ExternalInput").ap()
    wkT = nc.dram_tensor("wkT", [2, P, HD], BF16, kind="ExternalInput").ap()
    wvT = nc.dram_tensor("wvT", [2, P, HD], BF16, kind="ExternalInput").ap()
    woT = nc.dram_tensor("woT", [HD, C], BF16, kind="ExternalInput").ap()
    bq = nc.dram_tensor("bq", [HD, 1], F32, kind="ExternalInput").ap()
    bo = nc.dram_tensor("bo", [2, P, 1], F32, kind="ExternalInput").ap()
    y = nc.dram_tensor("y", [2, P, S], F32, kind="ExternalOutput").ap()

    with (
        tc.tile_pool(name="const", bufs=1) as cpool,
        tc.tile_pool(name="xp", bufs=1) as xpool,
        tc.tile_pool(name="qkv", bufs=1) as qpool,
        tc.tile_pool(name="es", bufs=3) as epool,
        tc.tile_pool(name="epi", bufs=2) as fpool,
        tc.tile_pool(name="ps", bufs=2, space="PSUM") as pp,
    ):
        # ---- weights / constants into SBUF ----
        wq_sb = cpool.tile([P, 2 * HD], BF16)
        wk_sb = cpool.tile([P, 2 * HD], BF16)
        wv_sb = cpool.tile([P, 2 * HD], BF16)
        for cch in range(2):
            nc.sync.dma_start(wq_sb[:, cch * HD:(cch + 1) * HD], wqT[cch])
            nc.sync.dma_start(wk_sb[:, cch * HD:(cch + 1) * HD], wkT[cch])
            nc.sync.dma_start(wv_sb[:, cch * HD:(cch + 1) * HD], wvT[cch])
        wo_sb = cpool.tile([HD, C], BF16)
        nc.sync.dma_start(wo_sb[:], woT)
        bq_sb = cpool.tile([HD, 1], F32)
        nc.sync.dma_start(bq_sb[:], bq)
        bo_sb = cpool.tile([P, 2], F32)
        for oh in range(2):
            nc.sync.dma_start(bo_sb[:, oh:oh + 1], bo[oh])
        ones_sb = cpool.tile([P, HD], BF16)
        nc.vector.memset(ones_sb[:], 1.0)

        # ---- activations into SBUF ----
        xq_sb = [xpool.tile([P, S], BF16, tag=f"xq{i}", name=f"xq_sb{i}")
                 for i in range(2)]
        xkv_sb = [xpool.tile([P, S], BF16, tag=f"xkv{i}", name=f"xkv_sb{i}")
                  for i in range(2)]
        for cch in range(2):
            nc.sync.dma_start(xq_sb[cch][:], xq[cch])
            nc.sync.dma_start(xkv_sb[cch][:], xkv[cch])

        q_sb = qpool.tile([HD, S], BF16)
        k_sb = qpool.tile([HD, S], BF16)
        # v^T with a ones column appended: [j-block partitions, (block, hd+1)]
        va_sb = qpool.tile([P, NJ * (HD + 1)], BF16)
        nc.vector.memset(va_sb[:], 1.0)

        # ---- q / k projections: [hd, S] = w^T.T @ x ----
        for t in range(S // 512):
            sl = slice(t * 512, (t + 1) * 512)
            qp = pp.tile([HD, 512], F32, tag="s", bufs=3)
            nc.tensor.matmul(qp[:], wq_sb[:, 0:HD], xq_sb[0][:, sl],
                             start=True, stop=False)
            nc.tensor.matmul(qp[:], wq_sb[:, HD:2 * HD], xq_sb[1][:, sl],
                             start=False, stop=True)
            nc.vector.tensor_scalar_add(q_sb[:, sl], qp[:], bq_sb[:])

            kp = pp.tile([HD, 512], F32, tag="s", bufs=3)
            nc.tensor.matmul(kp[:], wk_sb[:, 0:HD], xkv_sb[0][:, sl],
                             start=True, stop=False)
            nc.tensor.matmul(kp[:], wk_sb[:, HD:2 * HD], xkv_sb[1][:, sl],
                             start=False, stop=True)
            nc.vector.tensor_copy(k_sb[:, sl], kp[:])

        # ---- v^T projection: [j, hd] = x_kv(chunk).T @ wv^T(chunk) ----
        for j in range(NJ):
            jb = slice(j * P, (j + 1) * P)
            vp = pp.tile([P, HD], F32, tag="s", bufs=3)
            nc.tensor.matmul(vp[:], xkv_sb[0][:, jb], wv_sb[:, 0:HD],
                             start=True, stop=False)
            nc.tensor.matmul(vp[:], xkv_sb[1][:, jb], wv_sb[:, HD:2 * HD],
                             start=False, stop=True)
            base = j * (HD + 1)
            nc.vector.tensor_copy(va_sb[:, base:base + HD], vp[:])

        # ---- attention, i-chunk at a time ----
        for i in range(NI):
            av = pp.tile([HD + 1, IC], F32, tag="av", bufs=1)
            for j in range(NJ):
                jb = slice(j * P, (j + 1) * P)
                st = pp.tile([P, IC], F32, tag="s", bufs=3)
                for h in range(IC // 512):
                    isl = slice(i * IC + h * 512, i * IC + (h + 1) * 512)
                    nc.tensor.matmul(st[:, h * 512:(h + 1) * 512],
                                     k_sb[:, jb], q_sb[:, isl],
                                     start=True, stop=True)
                et = epool.tile([P, IC], BF16)
                for h in range(IC // EXP_W):
                    esl = slice(h * EXP_W, (h + 1) * EXP_W)
                    nc.scalar.activation(et[:, esl], st[:, esl],
                                         mybir.ActivationFunctionType.Exp)
                vbase = j * (HD + 1)
                for h in range(IC // 512):
                    nc.tensor.matmul(av[:, h * 512:(h + 1) * 512],
                                     va_sb[:, vbase:vbase + HD + 1],
                                     et[:, h * 512:(h + 1) * 512],
                                     start=(j == 0), stop=(j == NJ - 1))
                if JUNK_W:
                    # Dead matmul into its own rotating PSUM slot, never
                    # read. The loop is exp(ACT)-bound and the resulting
                    # ~25% PE idle makes the HW activity monitor throttle
                    # the PE to half clock, where it becomes the bottleneck
                    # and stays throttled; this pads PE duty to hold 2.4GHz.
                    jk = pp.tile([P, JUNK_W], F32, tag="s", bufs=3,
                                 name="jk")
                    nc.tensor.matmul(jk[:], k_sb[:, jb],
                                     q_sb[:, 0:JUNK_W],
                                     start=True, stop=True)

            # ---- epilogue: normalize and project out ----
            avs = fpool.tile([HD, IC], F32)
            nc.vector.tensor_copy(avs[:], av[0:HD, :])
            rcp = fpool.tile([HD + 1, IC], F32)
            nc.vector.reciprocal(rcp[HD:HD + 1, :], av[HD:HD + 1, :])
            rcpb = fpool.tile([HD + 1, IC], BF16)
            nc.vector.tensor_copy(rcpb[HD:HD + 1, :], rcp[HD:HD + 1, :])
            bcm = pp.tile([HD, IC], F32, tag="s", bufs=3)
            for h in range(IC // 512):
                nc.tensor.matmul(bcm[:, h * 512:(h + 1) * 512],
                                 ones_sb[HD:HD + 1, :],
                                 rcpb[HD:HD + 1, h * 512:(h + 1) * 512],
                                 start=True, stop=True)
            rcq = fpool.tile([HD, IC], F32)
            nc.vector.tensor_copy(rcq[:], bcm[:])
            outt = fpool.tile([HD, IC], BF16)
            nc.vector.tensor_mul(outt[:], avs[:], rcq[:])

            for oh in range(2):
                for h in range(IC // 512):
                    yp = pp.tile([P, 512], F32, tag="s", bufs=3)
                    nc.tensor.matmul(yp[:], wo_sb[:, oh * P:(oh + 1) * P],
                                     outt[:, h * 512:(h + 1) * 512],
                                     start=True, stop=True)
                    ys = fpool.tile([P, 512], F32)
                    nc.vector.tensor_scalar_add(ys[:], yp[:],
                                                bo_sb[:, oh:oh + 1])
                    nc.sync.dma_start(
                        y[oh][:, i * IC + h * 512:i * IC + (h + 1) * 512],
                        ys[:])


def build():
    nc = bacc.Bacc("TRN2", target_bir_lowering=False, debug=False,
                   enable_asserts=False)
    with tile.TileContext(nc) as tc:
        _emit(tc)
    nc.compile()
    return nc


_NC_CACHE = []


def _get_nc():
    if not _NC_CACHE:
        _NC_CACHE.append(build())
    return _NC_CACHE[0]


def make_in_maps(x_q, x_kv, wq, bq, wk, bk, wv, bv, wo, bo):
    bf = ml_dtypes.bfloat16
    in_maps = []
    for c in range(NCORES):
        b, n = divmod(c, NH)
        hs = slice(n * HD, (n + 1) * HD)
        wq_h = wq[hs].astype(np.float64) * SCALE
        bo_eff = wo[:, hs].astype(np.float64) @ bv[hs].astype(np.float64)
        if n == 0:
            bo_eff = bo_eff + bo.astype(np.float64)
        in_maps.append({
            "xq": np.ascontiguousarray(
                x_q[b].reshape(C, S).reshape(2, P, S)).astype(bf),
            "xkv": np.ascontiguousarray(
                x_kv[b].reshape(C, S).reshape(2, P, S)).astype(bf),
            "wqT": np.ascontiguousarray(wq_h.T.reshape(2, P, HD)).astype(bf),
            "wkT": np.ascontiguousarray(
                wk[hs].T.reshape(2, P, HD)).astype(bf),
            "wvT": np.ascontiguousarray(
                wv[hs].T.reshape(2, P, HD)).astype(bf),
            "woT": np.ascontiguousarray(wo[:, hs].T).astype(bf),
            "bq": (bq[hs].astype(np.float64) * SCALE
                   ).astype(np.float32).reshape(HD, 1),
            "bo": bo_eff.astype(np.float32).reshape(2, P, 1),
        })
    return in_maps


def assemble_output(results):
    y = np.zeros((B, C, S), np.float32)
    for c in range(NCORES):
        b = c // NH
        y[b] += results[c]["y"].reshape(C, S).astype(np.float32)
    return y.reshape(B, C, HGT, WID)


def kernel(**inputs):
    nc = _get_nc()
    in_maps = make_in_maps(**inputs)
    res = run_bass_kernel_spmd(nc, in_maps, list(range(NCORES)))
    return assemble_output(res.results)


if __name__ == "__main__":
    nc = build()
    print("built + compiled ok")


# revision 14
# speedup vs baseline: 1.1233x; 1.1233x over previous
"""Cross-attention kernel for Trainium2, sharded over 8 NeuronCores.

Problem (per reference):
  q = wq @ x_q + bq ; k = wk @ x_kv + bk ; v = wv @ x_kv + bv   (1x1 convs)
  per head: attn = softmax(q^T k / sqrt(hd)) ; out = attn @ v^T
  y = wo @ out + bo

Sharding: core c -> (batch b = c // 4, head n = c % 4). Each core runs one
head's full attention and produces the partial output projection
y_part = wo[:, head] @ out_head; the host sums the 4 head partials per batch.

Device-side simplifications (all mathematically exact):
  * bk drops out entirely: a per-query constant shift of the logits cancels
    in softmax.
  * bv folds into the output bias: sum_j softmax_ij = 1, so v-bias
    contributes wo_col @ bv, added to bo on the host.
  * scale 1/8 folds into wq/bq on the host.
  * no max-subtraction: logits are ~N(0,1) (max |logit| < ~6), exp is safe
    in fp32.
  * softmax denominator comes from a ones-column appended to v^T in the AV
    matmul; normalization happens after AV on [64, S] instead of [S, S].

Layouts: logits are computed transposed, S^T[j, i] (k stationary, q moving),
so the exp'd tile feeds the AV matmul directly with j on partitions — no
transposes anywhere. v^T is produced directly by using x_kv chunks as the
stationary operand of the v projection.
"""

import numpy as np
import ml_dtypes

import concourse.bacc as bacc
import concourse.mybir as mybir
import concourse.tile as tile
from concourse.bass_utils import run_bass_kernel_spmd

F32 = mybir.dt.float32
BF16 = mybir.dt.bfloat16

B, C, HGT, WID = 2, 256, 64, 64
S = HGT * WID  # 4096 pixels
NH, HD = 4, 64
NCORES = 8
P = 128
IC = 1024  # i-chunk width (2 PSUM banks)
NI = S // IC  # 4
NJ = S // P  # 32 j-blocks
SCALE = HD ** -0.5
EXP_W = 1024  # free width of one exp instruction (must divide IC)
# Width of the dummy matmul issued each j-iteration into the unused
# partitions of the AV PSUM tile. The attention loop is ACT(exp)-bound, so
# the PE idles ~25% per iteration; the hardware activity monitor then
# down-clocks it to 1.2 GHz, at which point it becomes the bottleneck and
# stays throttled (measured: stuck at K=4/8 for 90% of the run). Padding
# the PE's duty cycle with a throwaway matmul keeps it at 2.4 GHz.
JUNK_W = 0


def _emit(tc):
    nc = tc.nc
    xq = nc.dram_tensor("xq", [2, P, S], BF16, kind="ExternalInput").ap()
    xkv = nc.dram_tensor("xkv", [2, P, S], BF16, kind="ExternalInput").ap()
    wqT = nc.dram_tensor("wqT", [2, P, HD], BF16, kind="ExternalInput").ap()
    wkT = nc.dram_tensor("wkT", [2, P, HD], BF16, kind="ExternalInput").ap()
    wvT = nc.dram_tensor("wvT", [2, P, HD], BF16, kind="ExternalInput").ap()
    woT = nc.dram_tensor("woT", [HD, C], BF16, kind="ExternalInput").ap()
    bq = nc.dram_tensor("bq", [HD, 1], F32, kind="ExternalInput").ap()
    bo = nc.dram_tensor("bo", [2, P, 1], F32, kind="ExternalInput").ap()
    y = nc.dram_tensor("y", [2, P, S], F32, kind="ExternalOutput").ap()

    with (
        tc.tile_pool(name="const", bufs=1) as cpool,
        tc.tile_pool(name="xp", bufs=1) as xpool,
        tc.tile_pool(name="qkv", bufs=1) as qpool,
        tc.tile_pool(name="es", bufs=3) as epool,
        tc.tile_pool(name="epi", bufs=2) as fpool,
        tc.tile_pool(name="ps", bufs=2, space="PSUM") as pp,
    ):
        # ---- weights / constants into SBUF ----
        wq_sb = cpool.tile([P, 2 * HD], BF16)
        wk_sb = cpool.tile([P, 2 * HD], BF16)
        wv_sb = cpool.tile([P, 2 * HD], BF16)
        for cch in range(2):
            nc.sync.dma_start(wq_sb[:, cch * HD:(cch + 1) * HD], wqT[cch])
            nc.sync.dma_start(wk_sb[:, cch * HD:(cch + 1) * HD], wkT[cch])
            nc.sync.dma_start(wv_sb[:, cch * HD:(cch + 1) * HD], wvT[cch])
        wo_sb = cpool.tile([HD, C], BF16)
        nc.sync.dma_start(wo_sb[:], woT)
        bq_sb = cpool.tile([HD, 1], F32)
        nc.sync.dma_start(bq_sb[:], bq)
        bo_sb = cpool.tile([P, 2], F32)
        for oh in range(2):
            nc.sync.dma_start(bo_sb[:, oh:oh + 1], bo[oh])
        ones_sb = cpool.tile([P, HD], BF16)
        nc.vector.memset(ones_sb[:], 1.0)

        # ---- activations into SBUF ----
        xq_sb = [xpool.tile([P, S], BF16, tag=f"xq{i}", name=f"xq_sb{i}")
                 for i in range(2)]
        xkv_sb = [xpool.tile([P, S], BF16, tag=f"xkv{i}", name=f"xkv_sb{i}")
                  for i in range(2)]
        for cch in range(2):
            nc.sync.dma_start(xq_sb[cch][:], xq[cch])
            nc.sync.dma_start(xkv_sb[cch][:], xkv[cch])

        q_sb = qpool.tile([HD, S], BF16)
        k_sb = qpool.tile([HD, S], BF16)
        # v^T with a ones column appended: [j-block partitions, (block, hd+1)]
        va_sb = qpool.tile([P, NJ * (HD + 1)], BF16)
        nc.vector.memset(va_sb[:], 1.0)

        # ---- q / k projections: [hd, S] = w^T.T @ x ----
        for t in range(S // 512):
            sl = slice(t * 512, (t + 1) * 512)
            qp = pp.tile([HD, 512], F32, tag="s", bufs=2)
            nc.tensor.matmul(qp[:], wq_sb[:, 0:HD], xq_sb[0][:, sl],
                             start=True, stop=False)
            nc.tensor.matmul(qp[:], wq_sb[:, HD:2 * HD], xq_sb[1][:, sl],
                             start=False, stop=True)
            nc.vector.tensor_scalar_add(q_sb[:, sl], qp[:], bq_sb[:])

            kp = pp.tile([HD, 512], F32, tag="s", bufs=2)
            nc.tensor.matmul(kp[:], wk_sb[:, 0:HD], xkv_sb[0][:, sl],
                             start=True, stop=False)
            nc.tensor.matmul(kp[:], wk_sb[:, HD:2 * HD], xkv_sb[1][:, sl],
                             start=False, stop=True)
            nc.vector.tensor_copy(k_sb[:, sl], kp[:])

        # ---- v^T projection: [j, hd] = x_kv(chunk).T @ wv^T(chunk) ----
        for j in range(NJ):
            jb = slice(j * P, (j + 1) * P)
            vp = pp.tile([P, HD], F32, tag="s", bufs=2)
            nc.tensor.matmul(vp[:], xkv_sb[0][:, jb], wv_sb[:, 0:HD],
                             start=True, stop=False)
            nc.tensor.matmul(vp[:], xkv_sb[1][:, jb], wv_sb[:, HD:2 * HD],
                             start=False, stop=True)
            base = j * (HD + 1)
            nc.vector.tensor_copy(va_sb[:, base:base + HD], vp[:])

        # ---- attention, i-chunk at a time ----
        for i in range(NI):
            av = pp.tile([HD + 1, IC], F32, tag="av", bufs=2)
            for j in range(NJ):
                jb = slice(j * P, (j + 1) * P)
                st = pp.tile([P, IC], F32, tag="s", bufs=2)
                for h in range(IC // 512):
                    isl = slice(i * IC + h * 512, i * IC + (h + 1) * 512)
                    nc.tensor.matmul(st[:, h * 512:(h + 1) * 512],
                                     k_sb[:, jb], q_sb[:, isl],
                                     start=True, stop=True)
                et = epool.tile([P, IC], BF16)
                for h in range(IC // EXP_W):
                    esl = slice(h * EXP_W, (h + 1) * EXP_W)
                    nc.scalar.activation(et[:, esl], st[:, esl],
                                         mybir.ActivationFunctionType.Exp)
                vbase = j * (HD + 1)
                for h in range(IC // 512):
                    nc.tensor.matmul(av[:, h * 512:(h + 1) * 512],
                                     va_sb[:, vbase:vbase + HD + 1],
                                     et[:, h * 512:(h + 1) * 512],
                                     start=(j == 0), stop=(j == NJ - 1))
                if JUNK_W:
                    # Dead matmul into its own rotating PSUM slot, never
                    # read. The loop is exp(ACT)-bound and the resulting
                    # ~25% PE idle makes the HW activity monitor throttle
                    # the PE to half clock, where it becomes the bottleneck
                    # and stays throttled; this pads PE duty to hold 2.4GHz.
                    jk = pp.tile([P, JUNK_W], F32, tag="s", bufs=3,
                                 name="jk")
                    nc.tensor.matmul(jk[:], k_sb[:, jb],
                                     q_sb[:, 0:JUNK_W],
                                     start=True, stop=True)

            # ---- epilogue: normalize and project out ----
            avs = fpool.tile([HD, IC], F32)
            nc.vector.tensor_copy(avs[:], av[0:HD, :])
            rcp = fpool.tile([HD + 1, IC], F32)
            nc.vector.reciprocal(rcp[HD:HD + 1, :], av[HD:HD + 1, :])
            rcpb = fpool.tile([HD + 1, IC], BF16)
            nc.vector.tensor_copy(rcpb[HD:HD + 1, :], rcp[HD:HD + 1, :])
            bcm = pp.tile([HD, IC], F32, tag="s", bufs=2)
            for h in range(IC // 512):
                nc.tensor.matmul(bcm[:, h * 512:(h + 1) * 512],
                                 ones_sb[HD:HD + 1, :],
                                 rcpb[HD:HD + 1, h * 512:(h + 1) * 512],
                                 start=True, stop=True)
            rcq = fpool.tile([HD, IC], F32)
            nc.vector.tensor_copy(rcq[:], bcm[:])
            outt = fpool.tile([HD, IC], BF16)
            nc.vector.tensor_mul(outt[:], avs[:], rcq[:])

            for oh in range(2):
                for h in range(IC // 512):
                    yp = pp.tile([P, 512], F32, tag="s", bufs=2)
                    nc.tensor.matmul(yp[:], wo_sb[:, oh * P:(oh + 1) * P],
                                     outt[:, h * 512:(h + 1) * 512],
                                     start=True, stop=True)
                    ys = fpool.tile([P, 512], F32)
                    nc.vector.tensor_scalar_add(ys[:], yp[:],
                                                bo_sb[:, oh:oh + 1])
                    nc.sync.dma_start(
                        y[oh][:, i * IC + h * 512:i * IC + (h + 1) * 512],
                        ys[:])


def build():
    nc = bacc.Bacc("TRN2", target_bir_lowering=False, debug=False,
                   enable_asserts=False)
    with tile.TileContext(nc) as tc:
        _emit(tc)
    nc.compile()
    return nc


_NC_CACHE = []


def _get_nc():
    if not _NC_CACHE:
        _NC_CACHE.append(build())
    return _NC_CACHE[0]


def make_in_maps(x_q, x_kv, wq, bq, wk, bk, wv, bv, wo, bo):
    bf = ml_dtypes.bfloat16
    in_maps = []
    for c in range(NCORES):
        b, n = divmod(c, NH)
        hs = slice(n * HD, (n + 1) * HD)
        wq_h = wq[hs].astype(np.float64) * SCALE
        bo_eff = wo[:, hs].astype(np.float64) @ bv[hs].astype(np.float64)
        if n == 0:
            bo_eff = bo_eff + bo.astype(np.float64)
        in_maps.append({
            "xq": np.ascontiguousarray(
                x_q[b].reshape(C, S).reshape(2, P, S)).astype(bf),
            "xkv": np.ascontiguousarray(
                x_kv[b].reshape(C, S).reshape(2, P, S)).astype(bf),
            "wqT": np.ascontiguousarray(wq_h.T.reshape(2, P, HD)).astype(bf),
            "wkT": np.ascontiguousarray(
                wk[hs].T.reshape(2, P, HD)).astype(bf),
            "wvT": np.ascontiguousarray(
                wv[hs].T.reshape(2, P, HD)).astype(bf),
            "woT": np.ascontiguousarray(wo[:, hs].T).astype(bf),
            "bq": (bq[hs].astype(np.float64) * SCALE
                   ).astype(np.float32).reshape(HD, 1),
            "bo": bo_eff.astype(np.float32).reshape(2, P, 1),
        })
    return in_maps


def assemble_output(results):
    y = np.zeros((B, C, S), np.float32)
    for c in range(NCORES):
        b = c // NH
        y[b] += results[c]["y"].reshape(C, S).astype(np.float32)
    return y.reshape(B, C, HGT, WID)


def kernel(**inputs):
    nc = _get_nc()
    in_maps = make_in_maps(**inputs)
    res = run_bass_kernel_spmd(nc, in_maps, list(range(NCORES)))
    return assemble_output(res.results)


if __name__ == "__main__":
    nc = build()
    print("built + compiled ok")


# revision 16
# speedup vs baseline: 1.2634x; 1.1247x over previous
"""Cross-attention kernel for Trainium2, sharded over 8 NeuronCores.

Problem (per reference):
  q = wq @ x_q + bq ; k = wk @ x_kv + bk ; v = wv @ x_kv + bv   (1x1 convs)
  per head: attn = softmax(q^T k / sqrt(hd)) ; out = attn @ v^T
  y = wo @ out + bo

Sharding: core c -> (batch b = c // 4, head n = c % 4). Each core runs one
head's full attention and produces the partial output projection
y_part = wo[:, head] @ out_head; the host sums the 4 head partials per batch.

Device-side simplifications (all mathematically exact):
  * bk drops out entirely: a per-query constant shift of the logits cancels
    in softmax.
  * bv folds into the output bias: sum_j softmax_ij = 1, so v-bias
    contributes wo_col @ bv, added to bo on the host.
  * scale 1/8 folds into wq/bq on the host.
  * no max-subtraction: logits are ~N(0,1) (max |logit| < ~6), exp is safe
    in fp32.
  * softmax denominator comes from a ones-column appended to v^T in the AV
    matmul; normalization happens after AV on [64, S] instead of [S, S].

Layouts: logits are computed transposed, S^T[j, i] (k stationary, q moving),
so the exp'd tile feeds the AV matmul directly with j on partitions — no
transposes anywhere. v^T is produced directly by using x_kv chunks as the
stationary operand of the v projection.
"""

import numpy as np
import ml_dtypes

import concourse.bacc as bacc
import concourse.mybir as mybir
import concourse.tile as tile
from concourse.bass_utils import run_bass_kernel_spmd

F32 = mybir.dt.float32
BF16 = mybir.dt.bfloat16

B, C, HGT, WID = 2, 256, 64, 64
S = HGT * WID  # 4096 pixels
NH, HD = 4, 64
NCORES = 8
P = 128
IC = 1024  # i-chunk width (2 PSUM banks)
NI = S // IC  # 4
NJ = S // P  # 32 j-blocks
SCALE = HD ** -0.5
EXP_W = 1024  # free width of one exp instruction (must divide IC)
# Width of the dummy matmul issued each j-iteration into the unused
# partitions of the AV PSUM tile. The attention loop is ACT(exp)-bound, so
# the PE idles ~25% per iteration; the hardware activity monitor then
# down-clocks it to 1.2 GHz, at which point it becomes the bottleneck and
# stays throttled (measured: stuck at K=4/8 for 90% of the run). Padding
# the PE's duty cycle with a throwaway matmul keeps it at 2.4 GHz.
JUNK_W = 0


def _emit(tc):
    nc = tc.nc
    xq = nc.dram_tensor("xq", [2, P, S], BF16, kind="ExternalInput").ap()
    xkv = nc.dram_tensor("xkv", [2, P, S], BF16, kind="ExternalInput").ap()
    wqT = nc.dram_tensor("wqT", [2, P, HD], BF16, kind="ExternalInput").ap()
    wkT = nc.dram_tensor("wkT", [2, P, HD], BF16, kind="ExternalInput").ap()
    wvT = nc.dram_tensor("wvT", [2, P, HD], BF16, kind="ExternalInput").ap()
    woT = nc.dram_tensor("woT", [HD, C], BF16, kind="ExternalInput").ap()
    bq = nc.dram_tensor("bq", [HD, 1], F32, kind="ExternalInput").ap()
    bo = nc.dram_tensor("bo", [2, P, 1], F32, kind="ExternalInput").ap()
    y = nc.dram_tensor("y", [2, P, S], F32, kind="ExternalOutput").ap()

    with (
        tc.tile_pool(name="const", bufs=1) as cpool,
        tc.tile_pool(name="xp", bufs=1) as xpool,
        tc.tile_pool(name="qkv", bufs=1) as qpool,
        tc.tile_pool(name="es", bufs=3) as epool,
        tc.tile_pool(name="epi", bufs=2) as fpool,
        tc.tile_pool(name="ps", bufs=2, space="PSUM") as pp,
    ):
        # ---- weights / constants into SBUF ----
        wq_sb = cpool.tile([P, 2 * HD], BF16)
        wk_sb = cpool.tile([P, 2 * HD], BF16)
        wv_sb = cpool.tile([P, 2 * HD], BF16)
        for cch in range(2):
            nc.sync.dma_start(wq_sb[:, cch * HD:(cch + 1) * HD], wqT[cch])
            nc.sync.dma_start(wk_sb[:, cch * HD:(cch + 1) * HD], wkT[cch])
            nc.sync.dma_start(wv_sb[:, cch * HD:(cch + 1) * HD], wvT[cch])
        wo_sb = cpool.tile([HD, C], BF16)
        nc.sync.dma_start(wo_sb[:], woT)
        bq_sb = cpool.tile([HD, 1], F32)
        nc.sync.dma_start(bq_sb[:], bq)
        bo_sb = cpool.tile([P, 2], F32)
        for oh in range(2):
            nc.sync.dma_start(bo_sb[:, oh:oh + 1], bo[oh])
        ones_sb = cpool.tile([P, HD], BF16)
        nc.vector.memset(ones_sb[:], 1.0)
        # Zero bias for exp via memset: a float bias would become a DMA'd
        # const tensor whose transfer queues behind the 4MB input DMAs,
        # delaying the first exp (and idling the PE into a HAM down-clock).
        zbias_sb = cpool.tile([P, 1], F32)
        nc.vector.memset(zbias_sb[:], 0.0)
        # Warmup exp so the ~2.7us activation-table load happens during the
        # projection phase, not in front of the first real exp.
        warm_sb = cpool.tile([P, 1], BF16)
        nc.scalar.activation(warm_sb[:], zbias_sb[:],
                             mybir.ActivationFunctionType.Exp,
                             bias=zbias_sb[:])

        # ---- activations into SBUF ----
        xq_sb = [xpool.tile([P, S], BF16, tag=f"xq{i}", name=f"xq_sb{i}")
                 for i in range(2)]
        xkv_sb = [xpool.tile([P, S], BF16, tag=f"xkv{i}", name=f"xkv_sb{i}")
                  for i in range(2)]
        for cch in range(2):
            nc.sync.dma_start(xq_sb[cch][:], xq[cch])
            nc.sync.dma_start(xkv_sb[cch][:], xkv[cch])

        q_sb = qpool.tile([HD, S], BF16)
        k_sb = qpool.tile([HD, S], BF16)
        # v^T with a ones column appended: [j-block partitions, (block, hd+1)]
        va_sb = qpool.tile([P, NJ * (HD + 1)], BF16)
        nc.vector.memset(va_sb[:], 1.0)

        # ---- q / k projections: [hd, S] = w^T.T @ x ----
        for t in range(S // 512):
            sl = slice(t * 512, (t + 1) * 512)
            qp = pp.tile([HD, 512], F32, tag="s", bufs=2)
            nc.tensor.matmul(qp[:], wq_sb[:, 0:HD], xq_sb[0][:, sl],
                             start=True, stop=False)
            nc.tensor.matmul(qp[:], wq_sb[:, HD:2 * HD], xq_sb[1][:, sl],
                             start=False, stop=True)
            nc.vector.tensor_scalar_add(q_sb[:, sl], qp[:], bq_sb[:])

            kp = pp.tile([HD, 512], F32, tag="s", bufs=2)
            nc.tensor.matmul(kp[:], wk_sb[:, 0:HD], xkv_sb[0][:, sl],
                             start=True, stop=False)
            nc.tensor.matmul(kp[:], wk_sb[:, HD:2 * HD], xkv_sb[1][:, sl],
                             start=False, stop=True)
            nc.vector.tensor_copy(k_sb[:, sl], kp[:])

        # ---- v^T projection: [j, hd] = x_kv(chunk).T @ wv^T(chunk) ----
        for j in range(NJ):
            jb = slice(j * P, (j + 1) * P)
            vp = pp.tile([P, HD], F32, tag="s", bufs=2)
            nc.tensor.matmul(vp[:], xkv_sb[0][:, jb], wv_sb[:, 0:HD],
                             start=True, stop=False)
            nc.tensor.matmul(vp[:], xkv_sb[1][:, jb], wv_sb[:, HD:2 * HD],
                             start=False, stop=True)
            base = j * (HD + 1)
            nc.vector.tensor_copy(va_sb[:, base:base + HD], vp[:])

        # ---- attention, i-chunk at a time ----
        # The epilogue of chunk i is split: the DVE-only part (copy out of
        # PSUM + reciprocal) runs right after chunk i's j-loop; the PE part
        # (broadcast + out-projection matmuls) is deferred into the middle
        # of chunk i+1's j-loop. The PE executes in program order, so
        # emitting those matmuls at the chunk boundary would stall the PE
        # ~8us on the reciprocal chain — long enough for the HW activity
        # monitor to halve the PE clock for the rest of the kernel.
        pend = [None] * NI  # per chunk: (avs, rcp) awaiting part-2

        def epilogue_part2(i):
            avs, rcp = pend[i]
            rcpb = fpool.tile([HD + 1, IC], BF16, name="rcpb")
            nc.vector.tensor_copy(rcpb[HD:HD + 1, :], rcp[HD:HD + 1, :])
            bcm = pp.tile([HD, IC], F32, tag="av", bufs=2, name="bcm")
            for h in range(IC // 512):
                nc.tensor.matmul(bcm[:, h * 512:(h + 1) * 512],
                                 ones_sb[HD:HD + 1, :],
                                 rcpb[HD:HD + 1, h * 512:(h + 1) * 512],
                                 start=True, stop=True)
            rcq = fpool.tile([HD, IC], F32, name="rcq")
            nc.vector.tensor_copy(rcq[:], bcm[:])
            outt = fpool.tile([HD, IC], BF16, name="outt")
            nc.vector.tensor_mul(outt[:], avs[:], rcq[:])
            return outt

        def epilogue_part3(i, outt):
            for oh in range(2):
                for h in range(IC // 512):
                    yp = pp.tile([P, 512], F32, tag="av", bufs=2, name="yp")
                    nc.tensor.matmul(yp[:], wo_sb[:, oh * P:(oh + 1) * P],
                                     outt[:, h * 512:(h + 1) * 512],
                                     start=True, stop=True)
                    ys = fpool.tile([P, 512], F32, name="ys")
                    nc.vector.tensor_scalar_add(ys[:], yp[:],
                                                bo_sb[:, oh:oh + 1])
                    nc.sync.dma_start(
                        y[oh][:, i * IC + h * 512:i * IC + (h + 1) * 512],
                        ys[:])

        outt_prev = None
        for i in range(NI):
            av = pp.tile([HD + 1, IC], F32, tag="av", bufs=2)
            for j in range(NJ):
                if i > 0 and j == 8:
                    outt_prev = epilogue_part2(i - 1)
                if i > 0 and j == 16:
                    epilogue_part3(i - 1, outt_prev)
                jb = slice(j * P, (j + 1) * P)
                st = pp.tile([P, IC], F32, tag="s", bufs=2)
                for h in range(IC // 512):
                    isl = slice(i * IC + h * 512, i * IC + (h + 1) * 512)
                    nc.tensor.matmul(st[:, h * 512:(h + 1) * 512],
                                     k_sb[:, jb], q_sb[:, isl],
                                     start=True, stop=True)
                et = epool.tile([P, IC], BF16)
                for h in range(IC // EXP_W):
                    esl = slice(h * EXP_W, (h + 1) * EXP_W)
                    nc.scalar.activation(et[:, esl], st[:, esl],
                                         mybir.ActivationFunctionType.Exp,
                                         bias=zbias_sb[:])
                vbase = j * (HD + 1)
                for h in range(IC // 512):
                    nc.tensor.matmul(av[:, h * 512:(h + 1) * 512],
                                     va_sb[:, vbase:vbase + HD + 1],
                                     et[:, h * 512:(h + 1) * 512],
                                     start=(j == 0), stop=(j == NJ - 1))

            # epilogue part 1: DVE only — drain PSUM and free the av slot
            avs = fpool.tile([HD, IC], F32)
            nc.vector.tensor_copy(avs[:], av[0:HD, :])
            rcp = fpool.tile([HD + 1, IC], F32)
            nc.vector.reciprocal(rcp[HD:HD + 1, :], av[HD:HD + 1, :])
            pend[i] = (avs, rcp)

        epilogue_part3(NI - 1, epilogue_part2(NI - 1))


def build():
    nc = bacc.Bacc("TRN2", target_bir_lowering=False, debug=False,
                   enable_asserts=False)
    with tile.TileContext(nc) as tc:
        _emit(tc)
    nc.compile()
    return nc


_NC_CACHE = []


def _get_nc():
    if not _NC_CACHE:
        _NC_CACHE.append(build())
    return _NC_CACHE[0]


def make_in_maps(x_q, x_kv, wq, bq, wk, bk, wv, bv, wo, bo):
    bf = ml_dtypes.bfloat16
    in_maps = []
    for c in range(NCORES):
        b, n = divmod(c, NH)
        hs = slice(n * HD, (n + 1) * HD)
        wq_h = wq[hs].astype(np.float64) * SCALE
        bo_eff = wo[:, hs].astype(np.float64) @ bv[hs].astype(np.float64)
        if n == 0:
            bo_eff = bo_eff + bo.astype(np.float64)
        in_maps.append({
            "xq": np.ascontiguousarray(
                x_q[b].reshape(C, S).reshape(2, P, S)).astype(bf),
            "xkv": np.ascontiguousarray(
                x_kv[b].reshape(C, S).reshape(2, P, S)).astype(bf),
            "wqT": np.ascontiguousarray(wq_h.T.reshape(2, P, HD)).astype(bf),
            "wkT": np.ascontiguousarray(
                wk[hs].T.reshape(2, P, HD)).astype(bf),
            "wvT": np.ascontiguousarray(
                wv[hs].T.reshape(2, P, HD)).astype(bf),
            "woT": np.ascontiguousarray(wo[:, hs].T).astype(bf),
            "bq": (bq[hs].astype(np.float64) * SCALE
                   ).astype(np.float32).reshape(HD, 1),
            "bo": bo_eff.astype(np.float32).reshape(2, P, 1),
        })
    return in_maps


def assemble_output(results):
    y = np.zeros((B, C, S), np.float32)
    for c in range(NCORES):
        b = c // NH
        y[b] += results[c]["y"].reshape(C, S).astype(np.float32)
    return y.reshape(B, C, HGT, WID)


def kernel(**inputs):
    nc = _get_nc()
    in_maps = make_in_maps(**inputs)
    res = run_bass_kernel_spmd(nc, in_maps, list(range(NCORES)))
    return assemble_output(res.results)


if __name__ == "__main__":
    nc = build()
    print("built + compiled ok")


# revision 17
# speedup vs baseline: 1.3748x; 1.0882x over previous
"""Cross-attention kernel for Trainium2, sharded over 8 NeuronCores.

Problem (per reference):
  q = wq @ x_q + bq ; k = wk @ x_kv + bk ; v = wv @ x_kv + bv   (1x1 convs)
  per head: attn = softmax(q^T k / sqrt(hd)) ; out = attn @ v^T
  y = wo @ out + bo

Sharding: core c -> (batch b = c // 4, head n = c % 4). Each core runs one
head's full attention and produces the partial output projection
y_part = wo[:, head] @ out_head; the host sums the 4 head partials per batch.

Device-side simplifications (all mathematically exact):
  * bk drops out entirely: a per-query constant shift of the logits cancels
    in softmax.
  * bv folds into the output bias: sum_j softmax_ij = 1, so v-bias
    contributes wo_col @ bv, added to bo on the host.
  * scale 1/8 folds into wq/bq on the host.
  * no max-subtraction: logits are ~N(0,1) (max |logit| < ~6), exp is safe
    in fp32.
  * softmax denominator comes from a ones-column appended to v^T in the AV
    matmul; normalization happens after AV on [64, S] instead of [S, S].

Layouts: logits are computed transposed, S^T[j, i] (k stationary, q moving),
so the exp'd tile feeds the AV matmul directly with j on partitions — no
transposes anywhere. v^T is produced directly by using x_kv chunks as the
stationary operand of the v projection.
"""

import numpy as np
import ml_dtypes

import concourse.bacc as bacc
import concourse.mybir as mybir
import concourse.tile as tile
from concourse.bass_utils import run_bass_kernel_spmd

F32 = mybir.dt.float32
BF16 = mybir.dt.bfloat16

B, C, HGT, WID = 2, 256, 64, 64
S = HGT * WID  # 4096 pixels
NH, HD = 4, 64
NCORES = 8
P = 128
IC = 1024  # i-chunk width (2 PSUM banks)
NI = S // IC  # 4
NJ = S // P  # 32 j-blocks
SCALE = HD ** -0.5
EXP_W = 1024  # free width of one exp instruction (must divide IC)
# Width of the dummy matmul issued each j-iteration into the unused
# partitions of the AV PSUM tile. The attention loop is ACT(exp)-bound, so
# the PE idles ~25% per iteration; the hardware activity monitor then
# down-clocks it to 1.2 GHz, at which point it becomes the bottleneck and
# stays throttled (measured: stuck at K=4/8 for 90% of the run). Padding
# the PE's duty cycle with a throwaway matmul keeps it at 2.4 GHz.
JUNK_W = 0


def _emit(tc):
    nc = tc.nc
    xq = nc.dram_tensor("xq", [2, P, S], BF16, kind="ExternalInput").ap()
    xkv = nc.dram_tensor("xkv", [2, P, S], BF16, kind="ExternalInput").ap()
    wqT = nc.dram_tensor("wqT", [2, P, HD], BF16, kind="ExternalInput").ap()
    wkT = nc.dram_tensor("wkT", [2, P, HD], BF16, kind="ExternalInput").ap()
    wvT = nc.dram_tensor("wvT", [2, P, HD], BF16, kind="ExternalInput").ap()
    woT = nc.dram_tensor("woT", [HD, C], BF16, kind="ExternalInput").ap()
    bq = nc.dram_tensor("bq", [HD, 1], F32, kind="ExternalInput").ap()
    bo = nc.dram_tensor("bo", [2, P, 1], F32, kind="ExternalInput").ap()
    y = nc.dram_tensor("y", [2, P, S], F32, kind="ExternalOutput").ap()

    with (
        tc.tile_pool(name="const", bufs=1) as cpool,
        tc.tile_pool(name="xp", bufs=1) as xpool,
        tc.tile_pool(name="qkv", bufs=1) as qpool,
        tc.tile_pool(name="es", bufs=3) as epool,
        tc.tile_pool(name="epi", bufs=2) as fpool,
        tc.tile_pool(name="ps", bufs=2, space="PSUM") as pp,
    ):
        # ---- weights / constants into SBUF ----
        wq_sb = cpool.tile([P, 2 * HD], BF16)
        wk_sb = cpool.tile([P, 2 * HD], BF16)
        wv_sb = cpool.tile([P, 2 * HD], BF16)
        for cch in range(2):
            nc.sync.dma_start(wq_sb[:, cch * HD:(cch + 1) * HD], wqT[cch])
            nc.sync.dma_start(wk_sb[:, cch * HD:(cch + 1) * HD], wkT[cch])
            nc.sync.dma_start(wv_sb[:, cch * HD:(cch + 1) * HD], wvT[cch])
        wo_sb = cpool.tile([HD, C], BF16)
        nc.sync.dma_start(wo_sb[:], woT)
        bq_sb = cpool.tile([HD, 1], F32)
        nc.sync.dma_start(bq_sb[:], bq)
        bo_sb = cpool.tile([P, 2], F32)
        for oh in range(2):
            nc.sync.dma_start(bo_sb[:, oh:oh + 1], bo[oh])
        ones_sb = cpool.tile([P, HD], BF16)
        nc.vector.memset(ones_sb[:], 1.0)
        # Zero bias for exp via memset: a float bias would become a DMA'd
        # const tensor whose transfer queues behind the 4MB input DMAs,
        # delaying the first exp (and idling the PE into a HAM down-clock).
        zbias_sb = cpool.tile([P, 1], F32)
        nc.vector.memset(zbias_sb[:], 0.0)
        # Warmup exp so the ~2.7us activation-table load happens during the
        # projection phase, not in front of the first real exp.
        warm_sb = cpool.tile([P, 1], BF16)
        nc.scalar.activation(warm_sb[:], zbias_sb[:],
                             mybir.ActivationFunctionType.Exp,
                             bias=zbias_sb[:])

        # ---- activations into SBUF ----
        xq_sb = [xpool.tile([P, S], BF16, tag=f"xq{i}", name=f"xq_sb{i}")
                 for i in range(2)]
        xkv_sb = [xpool.tile([P, S], BF16, tag=f"xkv{i}", name=f"xkv_sb{i}")
                  for i in range(2)]
        # x_kv lands first (k and v^T projections run first); quarter-DMAs
        # let the projections start before the whole tensor arrives.
        QW = S // 4
        for cch in range(2):
            for qt in range(4):
                qsl = slice(qt * QW, (qt + 1) * QW)
                nc.sync.dma_start(xkv_sb[cch][:, qsl], xkv[cch][:, qsl])
        for cch in range(2):
            for qt in range(4):
                qsl = slice(qt * QW, (qt + 1) * QW)
                nc.sync.dma_start(xq_sb[cch][:, qsl], xq[cch][:, qsl])

        q_sb = qpool.tile([HD, S], BF16)
        k_sb = qpool.tile([HD, S], BF16)
        # v^T with a ones column appended: [j-block partitions, (block, hd+1)]
        va_sb = qpool.tile([P, NJ * (HD + 1)], BF16)
        nc.vector.memset(va_sb[:], 1.0)

        # ---- k projection: [hd, S] = wk^T.T @ x_kv ----
        for t in range(S // 512):
            sl = slice(t * 512, (t + 1) * 512)
            kp = pp.tile([HD, 512], F32, tag="s", bufs=2)
            nc.tensor.matmul(kp[:], wk_sb[:, 0:HD], xkv_sb[0][:, sl],
                             start=True, stop=False)
            nc.tensor.matmul(kp[:], wk_sb[:, HD:2 * HD], xkv_sb[1][:, sl],
                             start=False, stop=True)
            nc.vector.tensor_copy(k_sb[:, sl], kp[:])

        # ---- v^T projection: [j, hd] = x_kv(chunk).T @ wv^T(chunk) ----
        # (LDWEIGHTS-heavy / low PE duty: keep it away from the attention
        # loop so the activity monitor re-warms on the q projection below)
        for j in range(NJ):
            jb = slice(j * P, (j + 1) * P)
            vp = pp.tile([P, HD], F32, tag="s", bufs=2)
            nc.tensor.matmul(vp[:], xkv_sb[0][:, jb], wv_sb[:, 0:HD],
                             start=True, stop=False)
            nc.tensor.matmul(vp[:], xkv_sb[1][:, jb], wv_sb[:, HD:2 * HD],
                             start=False, stop=True)
            base = j * (HD + 1)
            nc.vector.tensor_copy(va_sb[:, base:base + HD], vp[:])

        # ---- q projection (scale and bias folded in on the host) ----
        for t in range(S // 512):
            sl = slice(t * 512, (t + 1) * 512)
            qp = pp.tile([HD, 512], F32, tag="s", bufs=2)
            nc.tensor.matmul(qp[:], wq_sb[:, 0:HD], xq_sb[0][:, sl],
                             start=True, stop=False)
            nc.tensor.matmul(qp[:], wq_sb[:, HD:2 * HD], xq_sb[1][:, sl],
                             start=False, stop=True)
            nc.vector.tensor_scalar_add(q_sb[:, sl], qp[:], bq_sb[:])

        # ---- attention, i-chunk at a time ----
        # The epilogue of chunk i is split: the DVE-only part (copy out of
        # PSUM + reciprocal) runs right after chunk i's j-loop; the PE part
        # (broadcast + out-projection matmuls) is deferred into the middle
        # of chunk i+1's j-loop. The PE executes in program order, so
        # emitting those matmuls at the chunk boundary would stall the PE
        # ~8us on the reciprocal chain — long enough for the HW activity
        # monitor to halve the PE clock for the rest of the kernel.
        pend = [None] * NI  # per chunk: (avs, rcp) awaiting part-2

        def epilogue_part2(i):
            avs, rcp = pend[i]
            rcpb = fpool.tile([HD + 1, IC], BF16, name="rcpb")
            nc.vector.tensor_copy(rcpb[HD:HD + 1, :], rcp[HD:HD + 1, :])
            bcm = pp.tile([HD, IC], F32, tag="av", bufs=2, name="bcm")
            for h in range(IC // 512):
                nc.tensor.matmul(bcm[:, h * 512:(h + 1) * 512],
                                 ones_sb[HD:HD + 1, :],
                                 rcpb[HD:HD + 1, h * 512:(h + 1) * 512],
                                 start=True, stop=True)
            rcq = fpool.tile([HD, IC], F32, name="rcq")
            nc.vector.tensor_copy(rcq[:], bcm[:])
            outt = fpool.tile([HD, IC], BF16, name="outt")
            nc.vector.tensor_mul(outt[:], avs[:], rcq[:])
            return outt

        def epilogue_part3(i, outt):
            for oh in range(2):
                for h in range(IC // 512):
                    yp = pp.tile([P, 512], F32, tag="av", bufs=2, name="yp")
                    nc.tensor.matmul(yp[:], wo_sb[:, oh * P:(oh + 1) * P],
                                     outt[:, h * 512:(h + 1) * 512],
                                     start=True, stop=True)
                    ys = fpool.tile([P, 512], F32, name="ys")
                    nc.vector.tensor_scalar_add(ys[:], yp[:],
                                                bo_sb[:, oh:oh + 1])
                    nc.sync.dma_start(
                        y[oh][:, i * IC + h * 512:i * IC + (h + 1) * 512],
                        ys[:])

        outt_prev = None
        for i in range(NI):
            av = pp.tile([HD + 1, IC], F32, tag="av", bufs=2)
            for j in range(NJ):
                if i > 0 and j == 8:
                    outt_prev = epilogue_part2(i - 1)
                if i > 0 and j == 16:
                    epilogue_part3(i - 1, outt_prev)
                jb = slice(j * P, (j + 1) * P)
                st = pp.tile([P, IC], F32, tag="s", bufs=2)
                for h in range(IC // 512):
                    isl = slice(i * IC + h * 512, i * IC + (h + 1) * 512)
                    nc.tensor.matmul(st[:, h * 512:(h + 1) * 512],
                                     k_sb[:, jb], q_sb[:, isl],
                                     start=True, stop=True)
                et = epool.tile([P, IC], BF16)
                for h in range(IC // EXP_W):
                    esl = slice(h * EXP_W, (h + 1) * EXP_W)
                    nc.scalar.activation(et[:, esl], st[:, esl],
                                         mybir.ActivationFunctionType.Exp,
                                         bias=zbias_sb[:])
                vbase = j * (HD + 1)
                for h in range(IC // 512):
                    nc.tensor.matmul(av[:, h * 512:(h + 1) * 512],
                                     va_sb[:, vbase:vbase + HD + 1],
                                     et[:, h * 512:(h + 1) * 512],
                                     start=(j == 0), stop=(j == NJ - 1))

            # epilogue part 1: DVE only — drain PSUM and free the av slot
            avs = fpool.tile([HD, IC], F32)
            nc.vector.tensor_copy(avs[:], av[0:HD, :])
            rcp = fpool.tile([HD + 1, IC], F32)
            nc.vector.reciprocal(rcp[HD:HD + 1, :], av[HD:HD + 1, :])
            pend[i] = (avs, rcp)

        epilogue_part3(NI - 1, epilogue_part2(NI - 1))


def build():
    nc = bacc.Bacc("TRN2", target_bir_lowering=False, debug=False,
                   enable_asserts=False)
    with tile.TileContext(nc) as tc:
        _emit(tc)
    nc.compile()
    return nc


_NC_CACHE = []


def _get_nc():
    if not _NC_CACHE:
        _NC_CACHE.append(build())
    return _NC_CACHE[0]


def make_in_maps(x_q, x_kv, wq, bq, wk, bk, wv, bv, wo, bo):
    bf = ml_dtypes.bfloat16
    in_maps = []
    for c in range(NCORES):
        b, n = divmod(c, NH)
        hs = slice(n * HD, (n + 1) * HD)
        wq_h = wq[hs].astype(np.float64) * SCALE
        bo_eff = wo[:, hs].astype(np.float64) @ bv[hs].astype(np.float64)
        if n == 0:
            bo_eff = bo_eff + bo.astype(np.float64)
        in_maps.append({
            "xq": np.ascontiguousarray(
                x_q[b].reshape(C, S).reshape(2, P, S)).astype(bf),
            "xkv": np.ascontiguousarray(
                x_kv[b].reshape(C, S).reshape(2, P, S)).astype(bf),
            "wqT": np.ascontiguousarray(wq_h.T.reshape(2, P, HD)).astype(bf),
            "wkT": np.ascontiguousarray(
                wk[hs].T.reshape(2, P, HD)).astype(bf),
            "wvT": np.ascontiguousarray(
                wv[hs].T.reshape(2, P, HD)).astype(bf),
            "woT": np.ascontiguousarray(wo[:, hs].T).astype(bf),
            "bq": (bq[hs].astype(np.float64) * SCALE
                   ).astype(np.float32).reshape(HD, 1),
            "bo": bo_eff.astype(np.float32).reshape(2, P, 1),
        })
    return in_maps


def assemble_output(results):
    y = np.zeros((B, C, S), np.float32)
    for c in range(NCORES):
        b = c // NH
        y[b] += results[c]["y"].reshape(C, S).astype(np.float32)
    return y.reshape(B, C, HGT, WID)


def kernel(**inputs):
    nc = _get_nc()
    in_maps = make_in_maps(**inputs)
    res = run_bass_kernel_spmd(nc, in_maps, list(range(NCORES)))
    return assemble_output(res.results)


if __name__ == "__main__":
    nc = build()
    print("built + compiled ok")


# revision 20
# speedup vs baseline: 1.3857x; 1.0079x over previous
"""Cross-attention kernel for Trainium2, sharded over 8 NeuronCores.

Problem (per reference):
  q = wq @ x_q + bq ; k = wk @ x_kv + bk ; v = wv @ x_kv + bv   (1x1 convs)
  per head: attn = softmax(q^T k / sqrt(hd)) ; out = attn @ v^T
  y = wo @ out + bo

Sharding: core c -> (batch b = c // 4, head n = c % 4). Each core runs one
head's full attention and produces the partial output projection
y_part = wo[:, head] @ out_head; the host sums the 4 head partials per batch.

Device-side simplifications (all mathematically exact):
  * bk drops out entirely: a per-query constant shift of the logits cancels
    in softmax.
  * bv folds into the output bias: sum_j softmax_ij = 1, so v-bias
    contributes wo_col @ bv, added to bo on the host.
  * scale 1/8 folds into wq/bq on the host.
  * no max-subtraction: logits are ~N(0,1) (max |logit| < ~6), exp is safe
    in fp32.
  * softmax denominator comes from a ones-column appended to v^T in the AV
    matmul; normalization happens after AV on [64, S] instead of [S, S].

Layouts: logits are computed transposed, S^T[j, i] (k stationary, q moving),
so the exp'd tile feeds the AV matmul directly with j on partitions — no
transposes anywhere. v^T is produced directly by using x_kv chunks as the
stationary operand of the v projection.
"""

import numpy as np
import ml_dtypes

import concourse.bacc as bacc
import concourse.mybir as mybir
import concourse.tile as tile
from concourse.bass_utils import run_bass_kernel_spmd

F32 = mybir.dt.float32
BF16 = mybir.dt.bfloat16

B, C, HGT, WID = 2, 256, 64, 64
S = HGT * WID  # 4096 pixels
NH, HD = 4, 64
NCORES = 8
P = 128
IC = 1024  # i-chunk width (2 PSUM banks)
NI = S // IC  # 4
NJ = S // P  # 32 j-blocks
SCALE = HD ** -0.5
EXP_W = 1024  # free width of one exp instruction (must divide IC)
# Width of the dummy matmul issued each j-iteration into the unused
# partitions of the AV PSUM tile. The attention loop is ACT(exp)-bound, so
# the PE idles ~25% per iteration; the hardware activity monitor then
# down-clocks it to 1.2 GHz, at which point it becomes the bottleneck and
# stays throttled (measured: stuck at K=4/8 for 90% of the run). Padding
# the PE's duty cycle with a throwaway matmul keeps it at 2.4 GHz.
JUNK_W = 0


def _emit(tc):
    nc = tc.nc
    xq = nc.dram_tensor("xq", [2, P, S], BF16, kind="ExternalInput").ap()
    xkv = nc.dram_tensor("xkv", [2, P, S], BF16, kind="ExternalInput").ap()
    wqT = nc.dram_tensor("wqT", [2, P, HD], BF16, kind="ExternalInput").ap()
    wkT = nc.dram_tensor("wkT", [2, P, HD], BF16, kind="ExternalInput").ap()
    wvT = nc.dram_tensor("wvT", [2, P, HD], BF16, kind="ExternalInput").ap()
    woT = nc.dram_tensor("woT", [HD, C], BF16, kind="ExternalInput").ap()
    bq = nc.dram_tensor("bq", [HD, 1], F32, kind="ExternalInput").ap()
    bo = nc.dram_tensor("bo", [2, P, 1], F32, kind="ExternalInput").ap()
    y = nc.dram_tensor("y", [2, P, S], F32, kind="ExternalOutput").ap()

    with (
        tc.tile_pool(name="const", bufs=1) as cpool,
        tc.tile_pool(name="xp", bufs=1) as xpool,
        tc.tile_pool(name="qkv", bufs=1) as qpool,
        tc.tile_pool(name="es", bufs=3) as epool,
        tc.tile_pool(name="epi", bufs=2) as fpool,
        tc.tile_pool(name="ps", bufs=2, space="PSUM") as pp,
    ):
        # ---- weights / constants into SBUF ----
        wq_sb = cpool.tile([P, 2 * HD], BF16)
        wk_sb = cpool.tile([P, 2 * HD], BF16)
        wv_sb = cpool.tile([P, 2 * HD], BF16)
        for cch in range(2):
            nc.sync.dma_start(wq_sb[:, cch * HD:(cch + 1) * HD], wqT[cch])
            nc.sync.dma_start(wk_sb[:, cch * HD:(cch + 1) * HD], wkT[cch])
            nc.sync.dma_start(wv_sb[:, cch * HD:(cch + 1) * HD], wvT[cch])
        wo_sb = cpool.tile([HD, C], BF16)
        nc.sync.dma_start(wo_sb[:], woT)
        bq_sb = cpool.tile([HD, 1], F32)
        nc.sync.dma_start(bq_sb[:], bq)
        bo_sb = cpool.tile([P, 2], F32)
        for oh in range(2):
            nc.sync.dma_start(bo_sb[:, oh:oh + 1], bo[oh])
        ones_sb = cpool.tile([P, HD], BF16)
        nc.vector.memset(ones_sb[:], 1.0)
        # Zero bias for exp via memset: a float bias would become a DMA'd
        # const tensor whose transfer queues behind the 4MB input DMAs,
        # delaying the first exp (and idling the PE into a HAM down-clock).
        zbias_sb = cpool.tile([P, 1], F32)
        nc.vector.memset(zbias_sb[:], 0.0)
        # Warmup exp so the ~2.7us activation-table load happens during the
        # projection phase, not in front of the first real exp.
        warm_sb = cpool.tile([P, 1], BF16)
        nc.scalar.activation(warm_sb[:], zbias_sb[:],
                             mybir.ActivationFunctionType.Exp,
                             bias=zbias_sb[:])

        # ---- activations into SBUF ----
        xq_sb = [xpool.tile([P, S], BF16, tag=f"xq{i}", name=f"xq_sb{i}")
                 for i in range(2)]
        xkv_sb = [xpool.tile([P, S], BF16, tag=f"xkv{i}", name=f"xkv_sb{i}")
                  for i in range(2)]
        # x_kv lands first (k and v^T projections run first); quarter-DMAs
        # let the projections start before the whole tensor arrives.
        QW = S // 4
        for qt in range(4):
            qsl = slice(qt * QW, (qt + 1) * QW)
            for cch in range(2):
                nc.sync.dma_start(xkv_sb[cch][:, qsl], xkv[cch][:, qsl])
        # x_q goes on the second HWDGE ring (ACT engine) so both inputs
        # stream in parallel.
        for qt in range(4):
            qsl = slice(qt * QW, (qt + 1) * QW)
            for cch in range(2):
                nc.scalar.dma_start(xq_sb[cch][:, qsl], xq[cch][:, qsl])

        q_sb = qpool.tile([HD, S], BF16)
        k_sb = qpool.tile([HD, S], BF16)
        # v^T with a ones column appended: [j-block partitions, (block, hd+1)]
        va_sb = qpool.tile([P, NJ * (HD + 1)], BF16)
        nc.vector.memset(va_sb[:], 1.0)

        # ---- v^T projection: [j, hd] = x_kv(chunk).T @ wv^T(chunk) ----
        # (LDWEIGHTS-heavy / low PE duty: run it first, while input DMAs are
        # still streaming and the PE is cold anyway; the dense k/q
        # projections below then warm the activity monitor right before the
        # attention loop)
        for j in range(NJ):
            jb = slice(j * P, (j + 1) * P)
            vp = pp.tile([P, HD], F32, tag="s", bufs=2)
            nc.tensor.matmul(vp[:], xkv_sb[0][:, jb], wv_sb[:, 0:HD],
                             start=True, stop=False)
            nc.tensor.matmul(vp[:], xkv_sb[1][:, jb], wv_sb[:, HD:2 * HD],
                             start=False, stop=True)
            base = j * (HD + 1)
            nc.vector.tensor_copy(va_sb[:, base:base + HD], vp[:])

        # ---- k projection: [hd, S] = wk^T.T @ x_kv ----
        for t in range(S // 512):
            sl = slice(t * 512, (t + 1) * 512)
            kp = pp.tile([HD, 512], F32, tag="s", bufs=2)
            nc.tensor.matmul(kp[:], wk_sb[:, 0:HD], xkv_sb[0][:, sl],
                             start=True, stop=False)
            nc.tensor.matmul(kp[:], wk_sb[:, HD:2 * HD], xkv_sb[1][:, sl],
                             start=False, stop=True)
            nc.vector.tensor_copy(k_sb[:, sl], kp[:])

        # ---- q projection (scale and bias folded in on the host) ----
        for t in range(S // 512):
            sl = slice(t * 512, (t + 1) * 512)
            qp = pp.tile([HD, 512], F32, tag="s", bufs=2)
            nc.tensor.matmul(qp[:], wq_sb[:, 0:HD], xq_sb[0][:, sl],
                             start=True, stop=False)
            nc.tensor.matmul(qp[:], wq_sb[:, HD:2 * HD], xq_sb[1][:, sl],
                             start=False, stop=True)
            nc.vector.tensor_scalar_add(q_sb[:, sl], qp[:], bq_sb[:])

        # ---- attention, i-chunk at a time ----
        # The epilogue of chunk i is split: the DVE-only part (copy out of
        # PSUM + reciprocal) runs right after chunk i's j-loop; the PE part
        # (broadcast + out-projection matmuls) is deferred into the middle
        # of chunk i+1's j-loop. The PE executes in program order, so
        # emitting those matmuls at the chunk boundary would stall the PE
        # ~8us on the reciprocal chain — long enough for the HW activity
        # monitor to halve the PE clock for the rest of the kernel.
        pend = [None] * NI  # per chunk: (avs, rcp) awaiting part-2

        def epilogue_part2(i):
            avs, rcp = pend[i]
            rcpb = fpool.tile([HD + 1, IC], BF16, name="rcpb")
            nc.vector.tensor_copy(rcpb[HD:HD + 1, :], rcp[HD:HD + 1, :])
            bcm = pp.tile([HD, IC], F32, tag="av", bufs=2, name="bcm")
            for h in range(IC // 512):
                nc.tensor.matmul(bcm[:, h * 512:(h + 1) * 512],
                                 ones_sb[HD:HD + 1, :],
                                 rcpb[HD:HD + 1, h * 512:(h + 1) * 512],
                                 start=True, stop=True)
            rcq = fpool.tile([HD, IC], F32, name="rcq")
            nc.vector.tensor_copy(rcq[:], bcm[:])
            outt = fpool.tile([HD, IC], BF16, name="outt")
            nc.vector.tensor_mul(outt[:], avs[:], rcq[:])
            return outt

        def epilogue_part3(i, outt):
            for oh in range(2):
                for h in range(IC // 512):
                    yp = pp.tile([P, 512], F32, tag="av", bufs=2, name="yp")
                    nc.tensor.matmul(yp[:], wo_sb[:, oh * P:(oh + 1) * P],
                                     outt[:, h * 512:(h + 1) * 512],
                                     start=True, stop=True)
                    ys = fpool.tile([P, 512], F32, name="ys")
                    nc.vector.tensor_scalar_add(ys[:], yp[:],
                                                bo_sb[:, oh:oh + 1])
                    nc.sync.dma_start(
                        y[oh][:, i * IC + h * 512:i * IC + (h + 1) * 512],
                        ys[:])

        outt_prev = None
        for i in range(NI):
            av = pp.tile([HD + 1, IC], F32, tag="av", bufs=2)
            for j in range(NJ):
                if i > 0 and j == 16:
                    outt_prev = epilogue_part2(i - 1)
                if i > 0 and j == 24:
                    epilogue_part3(i - 1, outt_prev)
                jb = slice(j * P, (j + 1) * P)
                st = pp.tile([P, IC], F32, tag="s", bufs=2)
                for h in range(IC // 512):
                    isl = slice(i * IC + h * 512, i * IC + (h + 1) * 512)
                    nc.tensor.matmul(st[:, h * 512:(h + 1) * 512],
                                     k_sb[:, jb], q_sb[:, isl],
                                     start=True, stop=True)
                et = epool.tile([P, IC], BF16)
                for h in range(IC // EXP_W):
                    esl = slice(h * EXP_W, (h + 1) * EXP_W)
                    nc.scalar.activation(et[:, esl], st[:, esl],
                                         mybir.ActivationFunctionType.Exp,
                                         bias=zbias_sb[:])
                vbase = j * (HD + 1)
                for h in range(IC // 512):
                    nc.tensor.matmul(av[:, h * 512:(h + 1) * 512],
                                     va_sb[:, vbase:vbase + HD + 1],
                                     et[:, h * 512:(h + 1) * 512],
                                     start=(j == 0), stop=(j == NJ - 1))

            # epilogue part 1: DVE only — drain PSUM and free the av slot
            avs = fpool.tile([HD, IC], F32)
            nc.vector.tensor_copy(avs[:], av[0:HD, :])
            rcp = fpool.tile([HD + 1, IC], F32)
            nc.vector.reciprocal(rcp[HD:HD + 1, :], av[HD:HD + 1, :])
            pend[i] = (avs, rcp)

        epilogue_part3(NI - 1, epilogue_part2(NI - 1))


def build():
    nc = bacc.Bacc("TRN2", target_bir_lowering=False, debug=False,
                   enable_asserts=False)
    with tile.TileContext(nc) as tc:
        _emit(tc)
    nc.compile()
    return nc


_NC_CACHE = []


def _get_nc():
    if not _NC_CACHE:
        _NC_CACHE.append(build())
    return _NC_CACHE[0]


def make_in_maps(x_q, x_kv, wq, bq, wk, bk, wv, bv, wo, bo):
    bf = ml_dtypes.bfloat16
    in_maps = []
    for c in range(NCORES):
        b, n = divmod(c, NH)
        hs = slice(n * HD, (n + 1) * HD)
        wq_h = wq[hs].astype(np.float64) * SCALE
        bo_eff = wo[:, hs].astype(np.float64) @ bv[hs].astype(np.float64)
        if n == 0:
            bo_eff = bo_eff + bo.astype(np.float64)
        in_maps.append({
            "xq": np.ascontiguousarray(
                x_q[b].reshape(C, S).reshape(2, P, S)).astype(bf),
            "xkv": np.ascontiguousarray(
                x_kv[b].reshape(C, S).reshape(2, P, S)).astype(bf),
            "wqT": np.ascontiguousarray(wq_h.T.reshape(2, P, HD)).astype(bf),
            "wkT": np.ascontiguousarray(
                wk[hs].T.reshape(2, P, HD)).astype(bf),
            "wvT": np.ascontiguousarray(
                wv[hs].T.reshape(2, P, HD)).astype(bf),
            "woT": np.ascontiguousarray(wo[:, hs].T).astype(bf),
            "bq": (bq[hs].astype(np.float64) * SCALE
                   ).astype(np.float32).reshape(HD, 1),
            "bo": bo_eff.astype(np.float32).reshape(2, P, 1),
        })
    return in_maps


def assemble_output(results):
    y = np.zeros((B, C, S), np.float32)
    for c in range(NCORES):
        b = c // NH
        y[b] += results[c]["y"].reshape(C, S).astype(np.float32)
    return y.reshape(B, C, HGT, WID)


def kernel(**inputs):
    nc = _get_nc()
    in_maps = make_in_maps(**inputs)
    res = run_bass_kernel_spmd(nc, in_maps, list(range(NCORES)))
    return assemble_output(res.results)


if __name__ == "__main__":
    nc = build()
    print("built + compiled ok")


# revision 24
# speedup vs baseline: 2.0153x; 1.4543x over previous
"""Cross-attention kernel for Trainium2, sharded over 8 NeuronCores.

Problem (per reference):
  q = wq @ x_q + bq ; k = wk @ x_kv + bk ; v = wv @ x_kv + bv   (1x1 convs)
  per head: attn = softmax(q^T k / sqrt(hd)) ; out = attn @ v^T
  y = wo @ out + bo

Sharding: core c -> (batch b = c // 4, head n = c % 4). Each core runs one
head's full attention and produces the partial output projection
y_part = wo[:, head] @ out_head; the host sums the 4 head partials per batch.

Device-side simplifications (all mathematically exact):
  * bk drops out entirely: a per-query constant shift of the logits cancels
    in softmax.
  * bv folds into the output bias: sum_j softmax_ij = 1, so v-bias
    contributes wo_col @ bv, added to bo on the host.
  * scale 1/8 folds into wq/bq on the host.
  * no max-subtraction: logits are ~N(0,1) (max |logit| < ~6), exp is safe
    in fp32.
  * softmax denominator comes from a ones-column appended to v^T in the AV
    matmul; normalization happens after AV on [64, S] instead of [S, S].

Layouts: logits are computed transposed, S^T[j, i] (k stationary, q moving),
so the exp'd tile feeds the AV matmul directly with j on partitions — no
transposes anywhere. v^T is produced directly by using x_kv chunks as the
stationary operand of the v projection.
"""

import numpy as np
import ml_dtypes

import concourse.bacc as bacc
import concourse.mybir as mybir
import concourse.tile as tile
from concourse.bass_utils import run_bass_kernel_spmd

F32 = mybir.dt.float32
BF16 = mybir.dt.bfloat16

B, C, HGT, WID = 2, 256, 64, 64
S = HGT * WID  # 4096 pixels
NH, HD = 4, 64
NCORES = 8
P = 128
IC = 1024  # i-chunk width (2 PSUM banks)
NI = S // IC  # 4
NJ = S // P  # 32 j-blocks
SCALE = HD ** -0.5
EXP_W = 1024  # free width of one exp instruction (must divide IC)


def _emit(tc):
    nc = tc.nc
    xq = nc.dram_tensor("xq", [2, P, S], BF16, kind="ExternalInput").ap()
    xkv = nc.dram_tensor("xkv", [2, P, S], BF16, kind="ExternalInput").ap()
    wqT = nc.dram_tensor("wqT", [2, P, HD], BF16, kind="ExternalInput").ap()
    wkT = nc.dram_tensor("wkT", [2, P, HD], BF16, kind="ExternalInput").ap()
    wvT = nc.dram_tensor("wvT", [2, P, HD], BF16, kind="ExternalInput").ap()
    woT = nc.dram_tensor("woT", [HD, C], BF16, kind="ExternalInput").ap()
    bq = nc.dram_tensor("bq", [HD, 1], F32, kind="ExternalInput").ap()
    y = nc.dram_tensor("y", [2, P, S], F32, kind="ExternalOutput").ap()
    yden = nc.dram_tensor("yden", [1, S], F32, kind="ExternalOutput").ap()

    with (
        tc.tile_pool(name="const", bufs=1) as cpool,
        tc.tile_pool(name="xp", bufs=1) as xpool,
        tc.tile_pool(name="qkv", bufs=1) as qpool,
        tc.tile_pool(name="es", bufs=3) as epool,
        tc.tile_pool(name="epi", bufs=2) as fpool,
        tc.tile_pool(name="ps", bufs=2, space="PSUM") as pp,
    ):
        # ---- weights / constants into SBUF ----
        wq_sb = cpool.tile([P, 2 * HD], BF16)
        wk_sb = cpool.tile([P, 2 * HD], BF16)
        wv_sb = cpool.tile([P, 2 * HD], BF16)
        for cch in range(2):
            nc.sync.dma_start(wq_sb[:, cch * HD:(cch + 1) * HD], wqT[cch])
            nc.sync.dma_start(wk_sb[:, cch * HD:(cch + 1) * HD], wkT[cch])
            nc.sync.dma_start(wv_sb[:, cch * HD:(cch + 1) * HD], wvT[cch])
        wo_sb = cpool.tile([HD, C], BF16)
        nc.sync.dma_start(wo_sb[:], woT)
        bq_sb = cpool.tile([HD, 1], F32)
        nc.sync.dma_start(bq_sb[:], bq)
        # Zero bias for exp via memset: a float bias would become a DMA'd
        # const tensor whose transfer queues behind the 4MB input DMAs,
        # delaying the first exp (and idling the PE into a HAM down-clock).
        zbias_sb = cpool.tile([P, 1], F32)
        nc.vector.memset(zbias_sb[:], 0.0)
        # Warmup exp so the ~2.7us activation-table load happens during the
        # projection phase, not in front of the first real exp.
        warm_sb = cpool.tile([P, 1], BF16)
        nc.scalar.activation(warm_sb[:], zbias_sb[:],
                             mybir.ActivationFunctionType.Exp,
                             bias=zbias_sb[:])

        # ---- activations into SBUF ----
        xq_sb = [xpool.tile([P, S], BF16, tag=f"xq{i}", name=f"xq_sb{i}")
                 for i in range(2)]
        xkv_sb = [xpool.tile([P, S], BF16, tag=f"xkv{i}", name=f"xkv_sb{i}")
                  for i in range(2)]
        # x_kv lands first (k and v^T projections run first); quarter-DMAs
        # let the projections start before the whole tensor arrives.
        QW = S // 4
        for qt in range(4):
            qsl = slice(qt * QW, (qt + 1) * QW)
            for cch in range(2):
                nc.sync.dma_start(xkv_sb[cch][:, qsl], xkv[cch][:, qsl])
        # x_q goes on the second HWDGE ring (ACT engine) so both inputs
        # stream in parallel.
        for qt in range(4):
            qsl = slice(qt * QW, (qt + 1) * QW)
            for cch in range(2):
                nc.scalar.dma_start(xq_sb[cch][:, qsl], xq[cch][:, qsl])

        q_sb = qpool.tile([HD, S], BF16)
        k_sb = qpool.tile([HD, S], BF16)
        # v^T with a ones column appended: [j-block partitions, (block, hd+1)]
        va_sb = qpool.tile([P, NJ * (HD + 1)], BF16)
        nc.vector.memset(va_sb[:], 1.0)

        # ---- v^T projection: [j, hd] = x_kv(chunk).T @ wv^T(chunk) ----
        # (LDWEIGHTS-heavy / low PE duty: run it first, while input DMAs are
        # still streaming and the PE is cold anyway; the dense k/q
        # projections below then warm the activity monitor right before the
        # attention loop)
        for j in range(NJ):
            jb = slice(j * P, (j + 1) * P)
            vp = pp.tile([P, HD], F32, tag="s", bufs=2)
            nc.tensor.matmul(vp[:], xkv_sb[0][:, jb], wv_sb[:, 0:HD],
                             start=True, stop=False)
            nc.tensor.matmul(vp[:], xkv_sb[1][:, jb], wv_sb[:, HD:2 * HD],
                             start=False, stop=True)
            base = j * (HD + 1)
            nc.vector.tensor_copy(va_sb[:, base:base + HD], vp[:])

        # ---- k projection: [hd, S] = wk^T.T @ x_kv ----
        for t in range(S // 512):
            sl = slice(t * 512, (t + 1) * 512)
            kp = pp.tile([HD, 512], F32, tag="s", bufs=2)
            nc.tensor.matmul(kp[:], wk_sb[:, 0:HD], xkv_sb[0][:, sl],
                             start=True, stop=False)
            nc.tensor.matmul(kp[:], wk_sb[:, HD:2 * HD], xkv_sb[1][:, sl],
                             start=False, stop=True)
            nc.vector.tensor_copy(k_sb[:, sl], kp[:])

        # ---- q projection (scale and bias folded in on the host) ----
        for t in range(S // 512):
            sl = slice(t * 512, (t + 1) * 512)
            qp = pp.tile([HD, 512], F32, tag="s", bufs=2)
            nc.tensor.matmul(qp[:], wq_sb[:, 0:HD], xq_sb[0][:, sl],
                             start=True, stop=False)
            nc.tensor.matmul(qp[:], wq_sb[:, HD:2 * HD], xq_sb[1][:, sl],
                             start=False, stop=True)
            nc.vector.tensor_scalar_add(q_sb[:, sl], qp[:], bq_sb[:])

        # ---- attention, i-chunk at a time ----
        # Softmax normalization is deferred to the host: division by the
        # denominator commutes with the output projection, so the device
        # ships y_un = wo_col @ (exp(S^T)^T V)^T plus the per-pixel
        # denominators, and the host computes y_un / den + bias. This keeps
        # any long dependency chain (reciprocal etc.) out of the in-order PE
        # stream — a multi-us PE stall makes the HW activity monitor halve
        # the PE clock for the rest of the kernel.
        pend = [None] * NI  # per chunk: unnormalized out^T awaiting out-proj

        def epilogue_part2(i):
            # out-projection of the (unnormalized) attention output
            outt = pend[i]
            for oh in range(2):
                for h in range(IC // 512):
                    yp = pp.tile([P, 512], F32, tag="av", bufs=2, name="yp")
                    nc.tensor.matmul(yp[:], wo_sb[:, oh * P:(oh + 1) * P],
                                     outt[:, h * 512:(h + 1) * 512],
                                     start=True, stop=True)
                    ys = fpool.tile([P, 512], F32, name="ys")
                    nc.vector.tensor_copy(ys[:], yp[:])
                    nc.sync.dma_start(
                        y[oh][:, i * IC + h * 512:i * IC + (h + 1) * 512],
                        ys[:])

        for i in range(NI):
            av = pp.tile([HD + 1, IC], F32, tag="av", bufs=2)
            for j in range(NJ):
                if i > 0 and j == 8:
                    epilogue_part2(i - 1)
                jb = slice(j * P, (j + 1) * P)
                st = pp.tile([P, IC], F32, tag="s", bufs=2)
                for h in range(IC // 512):
                    isl = slice(i * IC + h * 512, i * IC + (h + 1) * 512)
                    nc.tensor.matmul(st[:, h * 512:(h + 1) * 512],
                                     k_sb[:, jb], q_sb[:, isl],
                                     start=True, stop=True)
                et = epool.tile([P, IC], BF16)
                for h in range(IC // EXP_W):
                    esl = slice(h * EXP_W, (h + 1) * EXP_W)
                    nc.scalar.activation(et[:, esl], st[:, esl],
                                         mybir.ActivationFunctionType.Exp,
                                         bias=zbias_sb[:])
                vbase = j * (HD + 1)
                for h in range(IC // 512):
                    nc.tensor.matmul(av[:, h * 512:(h + 1) * 512],
                                     va_sb[:, vbase:vbase + HD + 1],
                                     et[:, h * 512:(h + 1) * 512],
                                     start=(j == 0), stop=(j == NJ - 1))

            # epilogue part 1: DVE only — drain PSUM and free the av slot
            outt = fpool.tile([HD, IC], BF16)
            nc.vector.tensor_copy(outt[:], av[0:HD, :])
            den = fpool.tile([HD + 1, IC], F32, name="den")
            nc.vector.tensor_copy(den[HD:HD + 1, :], av[HD:HD + 1, :])
            nc.sync.dma_start(yden[:, i * IC:(i + 1) * IC],
                              den[HD:HD + 1, :])
            pend[i] = outt

        epilogue_part2(NI - 1)


def build():
    nc = bacc.Bacc("TRN2", target_bir_lowering=False, debug=False,
                   enable_asserts=False)
    with tile.TileContext(nc) as tc:
        _emit(tc)
    nc.compile()
    return nc


_NC_CACHE = []


def _get_nc():
    if not _NC_CACHE:
        _NC_CACHE.append(build())
    return _NC_CACHE[0]


def make_in_maps(x_q, x_kv, wq, bq, wk, bk, wv, bv, wo, bo):
    bf = ml_dtypes.bfloat16
    in_maps = []
    bo_effs = []
    for c in range(NCORES):
        b, n = divmod(c, NH)
        hs = slice(n * HD, (n + 1) * HD)
        wq_h = wq[hs].astype(np.float64) * SCALE
        bo_eff = wo[:, hs].astype(np.float64) @ bv[hs].astype(np.float64)
        if n == 0:
            bo_eff = bo_eff + bo.astype(np.float64)
        bo_effs.append(bo_eff.astype(np.float32))
        in_maps.append({
            "xq": np.ascontiguousarray(
                x_q[b].reshape(C, S).reshape(2, P, S)).astype(bf),
            "xkv": np.ascontiguousarray(
                x_kv[b].reshape(C, S).reshape(2, P, S)).astype(bf),
            "wqT": np.ascontiguousarray(wq_h.T.reshape(2, P, HD)).astype(bf),
            "wkT": np.ascontiguousarray(
                wk[hs].T.reshape(2, P, HD)).astype(bf),
            "wvT": np.ascontiguousarray(
                wv[hs].T.reshape(2, P, HD)).astype(bf),
            "woT": np.ascontiguousarray(wo[:, hs].T).astype(bf),
            "bq": (bq[hs].astype(np.float64) * SCALE
                   ).astype(np.float32).reshape(HD, 1),
        })
    return in_maps, bo_effs


def assemble_output(results, bo_effs):
    # y_core is the unnormalized head partial; divide by the softmax
    # denominator and add the (host-folded) bias here.
    y = np.zeros((B, C, S), np.float32)
    for c in range(NCORES):
        b = c // NH
        den = results[c]["yden"].reshape(1, S)
        y[b] += results[c]["y"].reshape(C, S) / den \
            + bo_effs[c].reshape(C, 1)
    return y.reshape(B, C, HGT, WID)


def kernel(**inputs):
    nc = _get_nc()
    in_maps, bo_effs = make_in_maps(**inputs)
    res = run_bass_kernel_spmd(nc, in_maps, list(range(NCORES)))
    return assemble_output(res.results, bo_effs)


if __name__ == "__main__":
    nc = build()
    print("built + compiled ok")


# revision 25
# speedup vs baseline: 2.0936x; 1.0389x over previous
"""Cross-attention kernel for Trainium2, sharded over 8 NeuronCores.

Problem (per reference):
  q = wq @ x_q + bq ; k = wk @ x_kv + bk ; v = wv @ x_kv + bv   (1x1 convs)
  per head: attn = softmax(q^T k / sqrt(hd)) ; out = attn @ v^T
  y = wo @ out + bo

Sharding: core c -> (batch b = c // 4, head n = c % 4). Each core runs one
head's full attention and produces the partial output projection
y_part = wo[:, head] @ out_head; the host sums the 4 head partials per batch.

Device-side simplifications (all mathematically exact):
  * bk drops out entirely: a per-query constant shift of the logits cancels
    in softmax.
  * bv folds into the output bias: sum_j softmax_ij = 1, so v-bias
    contributes wo_col @ bv, added to bo on the host.
  * scale 1/8 folds into wq/bq on the host.
  * no max-subtraction: logits are ~N(0,1) (max |logit| < ~6), exp is safe
    in fp32.
  * softmax denominator comes from a ones-column appended to v^T in the AV
    matmul; normalization happens after AV on [64, S] instead of [S, S].

Layouts: logits are computed transposed, S^T[j, i] (k stationary, q moving),
so the exp'd tile feeds the AV matmul directly with j on partitions — no
transposes anywhere. v^T is produced directly by using x_kv chunks as the
stationary operand of the v projection.
"""

import numpy as np
import ml_dtypes

import concourse.bacc as bacc
import concourse.mybir as mybir
import concourse.tile as tile
from concourse.bass_utils import run_bass_kernel_spmd

F32 = mybir.dt.float32
BF16 = mybir.dt.bfloat16

B, C, HGT, WID = 2, 256, 64, 64
S = HGT * WID  # 4096 pixels
NH, HD = 4, 64
NCORES = 8
P = 128
IC = 1024  # i-chunk width (2 PSUM banks)
NI = S // IC  # 4
NJ = S // P  # 32 j-blocks
SCALE = HD ** -0.5
EXP_W = 1024  # free width of one exp instruction (must divide IC)


def _emit(tc):
    nc = tc.nc
    xq = nc.dram_tensor("xq", [2, P, S], BF16, kind="ExternalInput").ap()
    xkv = nc.dram_tensor("xkv", [2, P, S], BF16, kind="ExternalInput").ap()
    wqT = nc.dram_tensor("wqT", [2, P, HD], BF16, kind="ExternalInput").ap()
    wkT = nc.dram_tensor("wkT", [2, P, HD], BF16, kind="ExternalInput").ap()
    wvT = nc.dram_tensor("wvT", [2, P, HD], BF16, kind="ExternalInput").ap()
    woT = nc.dram_tensor("woT", [HD, C], BF16, kind="ExternalInput").ap()
    bq = nc.dram_tensor("bq", [HD, 1], F32, kind="ExternalInput").ap()
    y = nc.dram_tensor("y", [2, P, S], F32, kind="ExternalOutput").ap()
    yden = nc.dram_tensor("yden", [1, S], F32, kind="ExternalOutput").ap()

    with (
        tc.tile_pool(name="const", bufs=1) as cpool,
        tc.tile_pool(name="xp", bufs=1) as xpool,
        tc.tile_pool(name="qkv", bufs=1) as qpool,
        tc.tile_pool(name="es", bufs=3) as epool,
        tc.tile_pool(name="epi", bufs=2) as fpool,
        tc.tile_pool(name="ps", bufs=2, space="PSUM") as pp,
    ):
        # ---- weights / constants into SBUF ----
        wq_sb = cpool.tile([P, 2 * HD], BF16)
        wk_sb = cpool.tile([P, 2 * HD], BF16)
        wv_sb = cpool.tile([P, 2 * HD], BF16)
        for cch in range(2):
            nc.sync.dma_start(wq_sb[:, cch * HD:(cch + 1) * HD], wqT[cch])
            nc.sync.dma_start(wk_sb[:, cch * HD:(cch + 1) * HD], wkT[cch])
            nc.sync.dma_start(wv_sb[:, cch * HD:(cch + 1) * HD], wvT[cch])
        wo_sb = cpool.tile([HD, C], BF16)
        nc.sync.dma_start(wo_sb[:], woT)
        bq_sb = cpool.tile([HD, 1], F32)
        nc.sync.dma_start(bq_sb[:], bq)
        # Zero bias for exp via memset: a float bias would become a DMA'd
        # const tensor whose transfer queues behind the 4MB input DMAs,
        # delaying the first exp (and idling the PE into a HAM down-clock).
        zbias_sb = cpool.tile([P, 1], F32)
        nc.vector.memset(zbias_sb[:], 0.0)
        # Warmup exp so the ~2.7us activation-table load happens during the
        # projection phase, not in front of the first real exp.
        warm_sb = cpool.tile([P, 1], BF16)
        nc.scalar.activation(warm_sb[:], zbias_sb[:],
                             mybir.ActivationFunctionType.Exp,
                             bias=zbias_sb[:])

        # ---- activations into SBUF ----
        xq_sb = [xpool.tile([P, S], BF16, tag=f"xq{i}", name=f"xq_sb{i}")
                 for i in range(2)]
        xkv_sb = [xpool.tile([P, S], BF16, tag=f"xkv{i}", name=f"xkv_sb{i}")
                  for i in range(2)]
        # x_kv lands first (k and v^T projections run first); quarter-DMAs
        # let the projections start before the whole tensor arrives.
        QW = S // 4
        # Chunk 0 on the sync HWDGE ring, chunk 1 on the scalar-engine ring:
        # both rings stream in parallel, halving time-to-arrival.
        for qt in range(4):
            qsl = slice(qt * QW, (qt + 1) * QW)
            nc.sync.dma_start(xkv_sb[0][:, qsl], xkv[0][:, qsl])
            nc.scalar.dma_start(xkv_sb[1][:, qsl], xkv[1][:, qsl])
        for qt in range(4):
            qsl = slice(qt * QW, (qt + 1) * QW)
            nc.sync.dma_start(xq_sb[0][:, qsl], xq[0][:, qsl])
            nc.scalar.dma_start(xq_sb[1][:, qsl], xq[1][:, qsl])

        # q/k are zero-padded to 128 partitions and v^T blocks to 128
        # columns so every LDWEIGHTS is a full [128,128] bf16 load (fast
        # weight-load eligible); the zero rows contribute nothing.
        q_sb = qpool.tile([P, S], BF16)
        k_sb = qpool.tile([P, S], BF16)
        nc.vector.memset(q_sb[HD:P, :], 0.0)
        nc.vector.memset(k_sb[HD:P, :], 0.0)
        # v^T blocks: [j-part, (block, 128)]; col 64 = ones (denominator),
        # cols 65:128 zero.
        va_sb = qpool.tile([P, NJ * P], BF16)
        nc.vector.memset(va_sb[:], 0.0)
        va_v = va_sb.rearrange("p (j c) -> p j c", c=P)
        nc.vector.memset(va_v[:, :, HD:HD + 1], 1.0)

        # ---- v^T projection: [j, hd] = x_kv(chunk).T @ wv^T(chunk) ----
        # (LDWEIGHTS-heavy / low PE duty: run it first, while input DMAs are
        # still streaming and the PE is cold anyway; the dense k/q
        # projections below then warm the activity monitor right before the
        # attention loop)
        for j in range(NJ):
            jb = slice(j * P, (j + 1) * P)
            vp = pp.tile([P, HD], F32, tag="s", bufs=2)
            nc.tensor.matmul(vp[:], xkv_sb[0][:, jb], wv_sb[:, 0:HD],
                             start=True, stop=False)
            nc.tensor.matmul(vp[:], xkv_sb[1][:, jb], wv_sb[:, HD:2 * HD],
                             start=False, stop=True)
            base = j * P
            nc.vector.tensor_copy(va_sb[:, base:base + HD], vp[:])

        # ---- k projection: [hd, S] = wk^T.T @ x_kv ----
        for t in range(S // 512):
            sl = slice(t * 512, (t + 1) * 512)
            kp = pp.tile([HD, 512], F32, tag="s", bufs=2)
            nc.tensor.matmul(kp[:], wk_sb[:, 0:HD], xkv_sb[0][:, sl],
                             start=True, stop=False)
            nc.tensor.matmul(kp[:], wk_sb[:, HD:2 * HD], xkv_sb[1][:, sl],
                             start=False, stop=True)
            nc.vector.tensor_copy(k_sb[0:HD, sl], kp[:])

        # ---- q projection (scale and bias folded in on the host) ----
        for t in range(S // 512):
            sl = slice(t * 512, (t + 1) * 512)
            qp = pp.tile([HD, 512], F32, tag="s", bufs=2)
            nc.tensor.matmul(qp[:], wq_sb[:, 0:HD], xq_sb[0][:, sl],
                             start=True, stop=False)
            nc.tensor.matmul(qp[:], wq_sb[:, HD:2 * HD], xq_sb[1][:, sl],
                             start=False, stop=True)
            nc.vector.tensor_scalar_add(q_sb[0:HD, sl], qp[:], bq_sb[:])

        # ---- attention, i-chunk at a time ----
        # Softmax normalization is deferred to the host: division by the
        # denominator commutes with the output projection, so the device
        # ships y_un = wo_col @ (exp(S^T)^T V)^T plus the per-pixel
        # denominators, and the host computes y_un / den + bias. This keeps
        # any long dependency chain (reciprocal etc.) out of the in-order PE
        # stream — a multi-us PE stall makes the HW activity monitor halve
        # the PE clock for the rest of the kernel.
        pend = [None] * NI  # per chunk: unnormalized out^T awaiting out-proj

        def epilogue_part2(i):
            # out-projection of the (unnormalized) attention output
            outt = pend[i]
            for oh in range(2):
                for h in range(IC // 512):
                    yp = pp.tile([P, 512], F32, tag="av", bufs=2, name="yp")
                    nc.tensor.matmul(yp[:], wo_sb[:, oh * P:(oh + 1) * P],
                                     outt[:, h * 512:(h + 1) * 512],
                                     start=True, stop=True)
                    ys = fpool.tile([P, 512], F32, name="ys")
                    nc.vector.tensor_copy(ys[:], yp[:])
                    nc.sync.dma_start(
                        y[oh][:, i * IC + h * 512:i * IC + (h + 1) * 512],
                        ys[:])

        for i in range(NI):
            av = pp.tile([P, IC], F32, tag="av", bufs=2)
            for j in range(NJ):
                if i > 0 and j == 8:
                    epilogue_part2(i - 1)
                jb = slice(j * P, (j + 1) * P)
                st = pp.tile([P, IC], F32, tag="s", bufs=2)
                for h in range(IC // 512):
                    isl = slice(i * IC + h * 512, i * IC + (h + 1) * 512)
                    nc.tensor.matmul(st[:, h * 512:(h + 1) * 512],
                                     k_sb[:, jb], q_sb[:, isl],
                                     start=True, stop=True)
                et = epool.tile([P, IC], BF16)
                for h in range(IC // EXP_W):
                    esl = slice(h * EXP_W, (h + 1) * EXP_W)
                    nc.scalar.activation(et[:, esl], st[:, esl],
                                         mybir.ActivationFunctionType.Exp,
                                         bias=zbias_sb[:])
                vbase = j * P
                for h in range(IC // 512):
                    nc.tensor.matmul(av[:, h * 512:(h + 1) * 512],
                                     va_sb[:, vbase:vbase + P],
                                     et[:, h * 512:(h + 1) * 512],
                                     start=(j == 0), stop=(j == NJ - 1))

            # epilogue part 1: DVE only — drain PSUM and free the av slot
            outt = fpool.tile([HD, IC], BF16)
            nc.vector.tensor_copy(outt[:], av[0:HD, :])
            den = fpool.tile([HD + 1, IC], F32, name="den")
            nc.vector.tensor_copy(den[HD:HD + 1, :], av[HD:HD + 1, :])
            nc.sync.dma_start(yden[:, i * IC:(i + 1) * IC],
                              den[HD:HD + 1, :])
            pend[i] = outt

        epilogue_part2(NI - 1)


def build():
    nc = bacc.Bacc("TRN2", target_bir_lowering=False, debug=False,
                   enable_asserts=False)
    with tile.TileContext(nc) as tc:
        _emit(tc)
    nc.compile()
    return nc


_NC_CACHE = []


def _get_nc():
    if not _NC_CACHE:
        _NC_CACHE.append(build())
    return _NC_CACHE[0]


def make_in_maps(x_q, x_kv, wq, bq, wk, bk, wv, bv, wo, bo):
    bf = ml_dtypes.bfloat16
    in_maps = []
    bo_effs = []
    for c in range(NCORES):
        b, n = divmod(c, NH)
        hs = slice(n * HD, (n + 1) * HD)
        wq_h = wq[hs].astype(np.float64) * SCALE
        bo_eff = wo[:, hs].astype(np.float64) @ bv[hs].astype(np.float64)
        if n == 0:
            bo_eff = bo_eff + bo.astype(np.float64)
        bo_effs.append(bo_eff.astype(np.float32))
        in_maps.append({
            "xq": np.ascontiguousarray(
                x_q[b].reshape(C, S).reshape(2, P, S)).astype(bf),
            "xkv": np.ascontiguousarray(
                x_kv[b].reshape(C, S).reshape(2, P, S)).astype(bf),
            "wqT": np.ascontiguousarray(wq_h.T.reshape(2, P, HD)).astype(bf),
            "wkT": np.ascontiguousarray(
                wk[hs].T.reshape(2, P, HD)).astype(bf),
            "wvT": np.ascontiguousarray(
                wv[hs].T.reshape(2, P, HD)).astype(bf),
            "woT": np.ascontiguousarray(wo[:, hs].T).astype(bf),
            "bq": (bq[hs].astype(np.float64) * SCALE
                   ).astype(np.float32).reshape(HD, 1),
        })
    return in_maps, bo_effs


def assemble_output(results, bo_effs):
    # y_core is the unnormalized head partial; divide by the softmax
    # denominator and add the (host-folded) bias here.
    y = np.zeros((B, C, S), np.float32)
    for c in range(NCORES):
        b = c // NH
        den = results[c]["yden"].reshape(1, S)
        y[b] += results[c]["y"].reshape(C, S) / den \
            + bo_effs[c].reshape(C, 1)
    return y.reshape(B, C, HGT, WID)


def kernel(**inputs):
    nc = _get_nc()
    in_maps, bo_effs = make_in_maps(**inputs)
    res = run_bass_kernel_spmd(nc, in_maps, list(range(NCORES)))
    return assemble_output(res.results, bo_effs)


if __name__ == "__main__":
    nc = build()
    print("built + compiled ok")


# revision 26
# speedup vs baseline: 2.0973x; 1.0018x over previous
"""Cross-attention kernel for Trainium2, sharded over 8 NeuronCores.

Problem (per reference):
  q = wq @ x_q + bq ; k = wk @ x_kv + bk ; v = wv @ x_kv + bv   (1x1 convs)
  per head: attn = softmax(q^T k / sqrt(hd)) ; out = attn @ v^T
  y = wo @ out + bo

Sharding: core c -> (batch b = c // 4, head n = c % 4). Each core runs one
head's full attention and produces the partial output projection
y_part = wo[:, head] @ out_head; the host sums the 4 head partials per batch.

Device-side simplifications (all mathematically exact):
  * bk drops out entirely: a per-query constant shift of the logits cancels
    in softmax.
  * bv folds into the output bias: sum_j softmax_ij = 1, so v-bias
    contributes wo_col @ bv, added to bo on the host.
  * scale 1/8 folds into wq/bq on the host.
  * no max-subtraction: logits are ~N(0,1) (max |logit| < ~6), exp is safe
    in fp32.
  * softmax denominator comes from a ones-column appended to v^T in the AV
    matmul; normalization happens after AV on [64, S] instead of [S, S].

Layouts: logits are computed transposed, S^T[j, i] (k stationary, q moving),
so the exp'd tile feeds the AV matmul directly with j on partitions — no
transposes anywhere. v^T is produced directly by using x_kv chunks as the
stationary operand of the v projection.
"""

import numpy as np
import ml_dtypes

import concourse.bacc as bacc
import concourse.mybir as mybir
import concourse.tile as tile
from concourse.bass_utils import run_bass_kernel_spmd

F32 = mybir.dt.float32
BF16 = mybir.dt.bfloat16

B, C, HGT, WID = 2, 256, 64, 64
S = HGT * WID  # 4096 pixels
NH, HD = 4, 64
NCORES = 8
P = 128
IC = 1024  # i-chunk width (2 PSUM banks)
NI = S // IC  # 4
NJ = S // P  # 32 j-blocks
SCALE = HD ** -0.5
EXP_W = 1024  # free width of one exp instruction (must divide IC)


def _emit(tc):
    nc = tc.nc
    xq = nc.dram_tensor("xq", [2, P, S], BF16, kind="ExternalInput").ap()
    xkv = nc.dram_tensor("xkv", [2, P, S], BF16, kind="ExternalInput").ap()
    wqT = nc.dram_tensor("wqT", [2, P, HD], BF16, kind="ExternalInput").ap()
    wkT = nc.dram_tensor("wkT", [2, P, HD], BF16, kind="ExternalInput").ap()
    wvT = nc.dram_tensor("wvT", [2, P, HD], BF16, kind="ExternalInput").ap()
    woT = nc.dram_tensor("woT", [HD, C], BF16, kind="ExternalInput").ap()
    bq = nc.dram_tensor("bq", [HD, 1], F32, kind="ExternalInput").ap()
    y = nc.dram_tensor("y", [2, P, S], F32, kind="ExternalOutput").ap()
    yden = nc.dram_tensor("yden", [1, S], F32, kind="ExternalOutput").ap()

    with (
        tc.tile_pool(name="const", bufs=1) as cpool,
        tc.tile_pool(name="xp", bufs=1) as xpool,
        tc.tile_pool(name="qkv", bufs=1) as qpool,
        tc.tile_pool(name="es", bufs=3) as epool,
        tc.tile_pool(name="epi", bufs=2) as fpool,
        tc.tile_pool(name="ps", bufs=2, space="PSUM") as pp,
    ):
        # ---- weights / constants into SBUF ----
        wq_sb = cpool.tile([P, 2 * HD], BF16)
        wk_sb = cpool.tile([P, 2 * HD], BF16)
        wv_sb = cpool.tile([P, 2 * HD], BF16)
        # weights go via the gpsimd SWDGE queue — a third DMA path that
        # doesn't block the two HWDGE rings carrying the 4MB of activations
        for cch in range(2):
            nc.gpsimd.dma_start(wq_sb[:, cch * HD:(cch + 1) * HD], wqT[cch])
            nc.gpsimd.dma_start(wk_sb[:, cch * HD:(cch + 1) * HD], wkT[cch])
            nc.gpsimd.dma_start(wv_sb[:, cch * HD:(cch + 1) * HD], wvT[cch])
        wo_sb = cpool.tile([HD, C], BF16)
        nc.gpsimd.dma_start(wo_sb[:], woT)
        bq_sb = cpool.tile([HD, 1], F32)
        nc.gpsimd.dma_start(bq_sb[:], bq)
        # Zero bias for exp via memset: a float bias would become a DMA'd
        # const tensor whose transfer queues behind the 4MB input DMAs,
        # delaying the first exp (and idling the PE into a HAM down-clock).
        zbias_sb = cpool.tile([P, 1], F32)
        nc.vector.memset(zbias_sb[:], 0.0)
        # Warmup exp so the ~2.7us activation-table load happens during the
        # projection phase, not in front of the first real exp.
        warm_sb = cpool.tile([P, 1], BF16)
        nc.scalar.activation(warm_sb[:], zbias_sb[:],
                             mybir.ActivationFunctionType.Exp,
                             bias=zbias_sb[:])

        # ---- activations into SBUF ----
        xq_sb = [xpool.tile([P, S], BF16, tag=f"xq{i}", name=f"xq_sb{i}")
                 for i in range(2)]
        xkv_sb = [xpool.tile([P, S], BF16, tag=f"xkv{i}", name=f"xkv_sb{i}")
                  for i in range(2)]
        # x_kv lands first (k and v^T projections run first); quarter-DMAs
        # let the projections start before the whole tensor arrives.
        QW = S // 4
        # Chunk 0 on the sync HWDGE ring, chunk 1 on the scalar-engine ring:
        # both rings stream in parallel, halving time-to-arrival.
        for qt in range(4):
            qsl = slice(qt * QW, (qt + 1) * QW)
            nc.sync.dma_start(xkv_sb[0][:, qsl], xkv[0][:, qsl])
            nc.scalar.dma_start(xkv_sb[1][:, qsl], xkv[1][:, qsl])
        for qt in range(4):
            qsl = slice(qt * QW, (qt + 1) * QW)
            nc.sync.dma_start(xq_sb[0][:, qsl], xq[0][:, qsl])
            nc.scalar.dma_start(xq_sb[1][:, qsl], xq[1][:, qsl])

        # q/k are zero-padded to 128 partitions and v^T blocks to 128
        # columns so every LDWEIGHTS is a full [128,128] bf16 load (fast
        # weight-load eligible); the zero rows contribute nothing.
        q_sb = qpool.tile([P, S], BF16)
        k_sb = qpool.tile([P, S], BF16)
        nc.vector.memset(q_sb[HD:P, :], 0.0)
        nc.vector.memset(k_sb[HD:P, :], 0.0)
        # v^T blocks: [j-part, (block, 128)]; col 64 = ones (denominator),
        # cols 65:128 zero.
        va_sb = qpool.tile([P, NJ * P], BF16)
        nc.vector.memset(va_sb[:], 0.0)
        va_v = va_sb.rearrange("p (j c) -> p j c", c=P)
        nc.vector.memset(va_v[:, :, HD:HD + 1], 1.0)

        # ---- v^T projection: [j, hd] = x_kv(chunk).T @ wv^T(chunk) ----
        # (LDWEIGHTS-heavy / low PE duty: run it first, while input DMAs are
        # still streaming and the PE is cold anyway; the dense k/q
        # projections below then warm the activity monitor right before the
        # attention loop)
        for j in range(NJ):
            jb = slice(j * P, (j + 1) * P)
            vp = pp.tile([P, HD], F32, tag="s", bufs=2)
            nc.tensor.matmul(vp[:], xkv_sb[0][:, jb], wv_sb[:, 0:HD],
                             start=True, stop=False)
            nc.tensor.matmul(vp[:], xkv_sb[1][:, jb], wv_sb[:, HD:2 * HD],
                             start=False, stop=True)
            base = j * P
            nc.vector.tensor_copy(va_sb[:, base:base + HD], vp[:])

        # ---- k projection: [hd, S] = wk^T.T @ x_kv ----
        for t in range(S // 512):
            sl = slice(t * 512, (t + 1) * 512)
            kp = pp.tile([HD, 512], F32, tag="s", bufs=2)
            nc.tensor.matmul(kp[:], wk_sb[:, 0:HD], xkv_sb[0][:, sl],
                             start=True, stop=False)
            nc.tensor.matmul(kp[:], wk_sb[:, HD:2 * HD], xkv_sb[1][:, sl],
                             start=False, stop=True)
            nc.vector.tensor_copy(k_sb[0:HD, sl], kp[:])

        # ---- q projection (scale and bias folded in on the host) ----
        for t in range(S // 512):
            sl = slice(t * 512, (t + 1) * 512)
            qp = pp.tile([HD, 512], F32, tag="s", bufs=2)
            nc.tensor.matmul(qp[:], wq_sb[:, 0:HD], xq_sb[0][:, sl],
                             start=True, stop=False)
            nc.tensor.matmul(qp[:], wq_sb[:, HD:2 * HD], xq_sb[1][:, sl],
                             start=False, stop=True)
            nc.vector.tensor_scalar_add(q_sb[0:HD, sl], qp[:], bq_sb[:])

        # ---- attention, i-chunk at a time ----
        # Softmax normalization is deferred to the host: division by the
        # denominator commutes with the output projection, so the device
        # ships y_un = wo_col @ (exp(S^T)^T V)^T plus the per-pixel
        # denominators, and the host computes y_un / den + bias. This keeps
        # any long dependency chain (reciprocal etc.) out of the in-order PE
        # stream — a multi-us PE stall makes the HW activity monitor halve
        # the PE clock for the rest of the kernel.
        pend = [None] * NI  # per chunk: unnormalized out^T awaiting out-proj

        def epilogue_part2(i):
            # out-projection of the (unnormalized) attention output
            outt = pend[i]
            for oh in range(2):
                for h in range(IC // 512):
                    yp = pp.tile([P, 512], F32, tag="av", bufs=2, name="yp")
                    nc.tensor.matmul(yp[:], wo_sb[:, oh * P:(oh + 1) * P],
                                     outt[:, h * 512:(h + 1) * 512],
                                     start=True, stop=True)
                    ys = fpool.tile([P, 512], F32, name="ys")
                    nc.vector.tensor_copy(ys[:], yp[:])
                    eng = nc.sync if oh == 0 else nc.scalar
                    eng.dma_start(
                        y[oh][:, i * IC + h * 512:i * IC + (h + 1) * 512],
                        ys[:])

        for i in range(NI):
            av = pp.tile([P, IC], F32, tag="av", bufs=2)
            for j in range(NJ):
                if i > 0 and j == 8:
                    epilogue_part2(i - 1)
                jb = slice(j * P, (j + 1) * P)
                st = pp.tile([P, IC], F32, tag="s", bufs=2)
                for h in range(IC // 512):
                    isl = slice(i * IC + h * 512, i * IC + (h + 1) * 512)
                    nc.tensor.matmul(st[:, h * 512:(h + 1) * 512],
                                     k_sb[:, jb], q_sb[:, isl],
                                     start=True, stop=True)
                et = epool.tile([P, IC], BF16)
                for h in range(IC // EXP_W):
                    esl = slice(h * EXP_W, (h + 1) * EXP_W)
                    nc.scalar.activation(et[:, esl], st[:, esl],
                                         mybir.ActivationFunctionType.Exp,
                                         bias=zbias_sb[:])
                vbase = j * P
                for h in range(IC // 512):
                    nc.tensor.matmul(av[:, h * 512:(h + 1) * 512],
                                     va_sb[:, vbase:vbase + P],
                                     et[:, h * 512:(h + 1) * 512],
                                     start=(j == 0), stop=(j == NJ - 1))

            # epilogue part 1: DVE only — drain PSUM and free the av slot
            outt = fpool.tile([HD, IC], BF16)
            nc.vector.tensor_copy(outt[:], av[0:HD, :])
            den = fpool.tile([HD + 1, IC], F32, name="den")
            nc.vector.tensor_copy(den[HD:HD + 1, :], av[HD:HD + 1, :])
            nc.gpsimd.dma_start(yden[:, i * IC:(i + 1) * IC],
                                den[HD:HD + 1, :])
            pend[i] = outt

        epilogue_part2(NI - 1)


def build():
    nc = bacc.Bacc("TRN2", target_bir_lowering=False, debug=False,
                   enable_asserts=False)
    with tile.TileContext(nc) as tc:
        _emit(tc)
    nc.compile()
    return nc


_NC_CACHE = []


def _get_nc():
    if not _NC_CACHE:
        _NC_CACHE.append(build())
    return _NC_CACHE[0]


def make_in_maps(x_q, x_kv, wq, bq, wk, bk, wv, bv, wo, bo):
    bf = ml_dtypes.bfloat16
    in_maps = []
    bo_effs = []
    for c in range(NCORES):
        b, n = divmod(c, NH)
        hs = slice(n * HD, (n + 1) * HD)
        wq_h = wq[hs].astype(np.float64) * SCALE
        bo_eff = wo[:, hs].astype(np.float64) @ bv[hs].astype(np.float64)
        if n == 0:
            bo_eff = bo_eff + bo.astype(np.float64)
        bo_effs.append(bo_eff.astype(np.float32))
        in_maps.append({
            "xq": np.ascontiguousarray(
                x_q[b].reshape(C, S).reshape(2, P, S)).astype(bf),
            "xkv": np.ascontiguousarray(
                x_kv[b].reshape(C, S).reshape(2, P, S)).astype(bf),
            "wqT": np.ascontiguousarray(wq_h.T.reshape(2, P, HD)).astype(bf),
            "wkT": np.ascontiguousarray(
                wk[hs].T.reshape(2, P, HD)).astype(bf),
            "wvT": np.ascontiguousarray(
                wv[hs].T.reshape(2, P, HD)).astype(bf),
            "woT": np.ascontiguousarray(wo[:, hs].T).astype(bf),
            "bq": (bq[hs].astype(np.float64) * SCALE
                   ).astype(np.float32).reshape(HD, 1),
        })
    return in_maps, bo_effs


def assemble_output(results, bo_effs):
    # y_core is the unnormalized head partial; divide by the softmax
    # denominator and add the (host-folded) bias here.
    y = np.zeros((B, C, S), np.float32)
    for c in range(NCORES):
        b = c // NH
        den = results[c]["yden"].reshape(1, S)
        y[b] += results[c]["y"].reshape(C, S) / den \
            + bo_effs[c].reshape(C, 1)
    return y.reshape(B, C, HGT, WID)


def kernel(**inputs):
    nc = _get_nc()
    in_maps, bo_effs = make_in_maps(**inputs)
    res = run_bass_kernel_spmd(nc, in_maps, list(range(NCORES)))
    return assemble_output(res.results, bo_effs)


if __name__ == "__main__":
    nc = build()
    print("built + compiled ok")
